# revision 32
# baseline (speedup 1.0000x reference)
"""Distributed Trainium2 Bass kernel for fused LayerNorm + causal multi-head
attention + output projection (B=2, T=2048, DIM=1024, H=16, D=64) on 8 cores.

Sharding (v6):
  - LayerNorm + QKV projection + final projection: token-parallel
    (512 tokens/core). QKV is computed on LOCAL data (full 3072-row weight)
    BEFORE any collective, so the first-collective rendezvous (launch skew)
    is absorbed by ~60us of real matmul work instead of idle waiting.
  - qkv travels via one bf16 AllToAll into head-parallel layout
    (2 heads x 2 batches per core); attention outputs return via a second
    bf16 AllToAll; projection is token-parallel again.
  - causal diagonal blocks are N-trimmed; triangular mask via precomputed
    bf16 multiply (DVE+Pool); denominators via the vnat ones-column trick.

Compute dtype: bf16 matmuls with fp32 PSUM accumulation (rel err ~5e-3).
LN affine params and the 1/sqrt(D) score scale are folded into the QKV
weights on the host.
"""
import os
import sys
import types
import numpy as np
import ml_dtypes

# ---------------------------------------------------------------- constants
B, T, DIM, D = 2, 2048, 1024, 64
H = DIM // D            # 16 heads
NC = 8                  # cores
TOK = B * T             # 4096 tokens
TPC = TOK // NC         # 512 tokens per core
KT8 = DIM // 128        # 8 contraction tiles
GT = 3 * DIM // 128     # 24 qkv output tiles of 128 rows
EPS = 1e-5

TRACE = bool(int(os.environ.get("BASS_KERNEL_TRACE", "0")))
DUMMY_QA2A = int(os.environ.get("DUMMY_QA2A", "110"))
DUMMY_A2A = int(os.environ.get("DUMMY_A2A", "85"))

BF16_NP = ml_dtypes.bfloat16


def _ensure_ntff_hook():
    """The agent image lacks antenv.axon_hooks; recreate it so trace=True works."""
    if "antenv.axon_hooks" not in sys.modules:
        mod = types.ModuleType("antenv.axon_hooks")
        mod._hook = None
        def set_axon_ntff_profile_hook(h):
            mod._hook = h
        def get_axon_ntff_profile_hook():
            return mod._hook
        mod.set_axon_ntff_profile_hook = set_axon_ntff_profile_hook
        mod.get_axon_ntff_profile_hook = get_axon_ntff_profile_hook
        sys.modules["antenv.axon_hooks"] = mod
    m = sys.modules["antenv.axon_hooks"]
    if m.get_axon_ntff_profile_hook() is None:
        try:
            from trn_agent_boot.trn_boot import _ntff_profile_via_ctypes
            m.set_axon_ntff_profile_hook(
                _ntff_profile_via_ctypes("/opt/axon/libaxon_pjrt.so"))
        except Exception:
            pass


def build_graph():
    import concourse.bass as bass
    import concourse.bacc as bacc
    import concourse.tile as tile
    import concourse.mybir as mybir

    dt = mybir.dt
    F32, BF16 = dt.float32, dt.bfloat16
    AF = mybir.ActivationFunctionType
    ALU = mybir.AluOpType
    RG = [list(range(NC))]

    nc = bacc.Bacc(None, target_bir_lowering=False, debug=False, num_devices=NC)

    # ------------------------------------------------------------ I/O
    x_in = nc.dram_tensor("x_c", [TPC, DIM], F32, kind="ExternalInput")
    wt_in = nc.dram_tensor("wt_c", [DIM, 3 * DIM], BF16, kind="ExternalInput")
    bias_in = nc.dram_tensor("bias_c", [128, GT], F32, kind="ExternalInput")
    pwt_in = nc.dram_tensor("pwt", [DIM, DIM], BF16, kind="ExternalInput")
    pbf_in = nc.dram_tensor("pbf", [128, DIM], BF16, kind="ExternalInput")
    idn_in = nc.dram_tensor("idn", [128, 128], BF16, kind="ExternalInput")
    tri_in = nc.dram_tensor("tri", [128, 512], BF16, kind="ExternalInput")
    emat_in = nc.dram_tensor("emat", [33, 128], BF16, kind="ExternalInput")
    out_dram = nc.dram_tensor("out_c", [TPC, DIM], F32, kind="ExternalOutput")

    with tile.TileContext(nc) as tc:
        with (
            tc.tile_pool(name="persist", bufs=1) as pers,
            tc.tile_pool(name="dram", bufs=1, space="DRAM") as dram,
        ):
            # ---------------- DRAM bounce buffers ----------------
            qa_in = dram.tile([NC * 384, TPC], BF16)          # qkv AllToAll
            qa_out = dram.tile([NC * 384, TPC], BF16)
            ao_in = dram.tile([NC * 128, TPC], BF16)          # attn-out AllToAll
            ao_out = dram.tile([NC * 128, TPC], BF16)

            # idn first: transposes need it early; it is tiny
            idn_sb = pers.tile([128, 128], BF16)
            nc.sync.dma_start(idn_sb[:], idn_in[:])

            # ================= P1: LayerNorm (token slice, natural) ========
            xn_sb = pers.tile([128, 4 * DIM], BF16)   # 4 token tiles side by side
            wt_sb = pers.tile([128, GT * DIM], BF16)  # gt-major, k-minor qkv weights
            with tc.tile_pool(name="ln", bufs=4) as lnp:
                # x tiles first on the DMA queue, then the 24 weight-tile DMAs
                xts = []
                for t in range(4):
                    xt = lnp.tile([128, DIM], F32, tag="xt", name=f"xt{t}")
                    nc.sync.dma_start(xt[:], x_in[128 * t:128 * (t + 1), :])
                    xts.append(xt)
                for gt in range(GT):
                    nc.sync.dma_start(
                        wt_sb[:, DIM * gt:DIM * (gt + 1)]
                        .rearrange("p (k o) -> p k o", o=128),
                        wt_in[:, 128 * gt:128 * (gt + 1)]
                        .rearrange("(k p) o -> p k o", p=128),
                    )
                for t in range(4):
                    xt = xts[t]
                    nmu = lnp.tile([128, 1], F32, tag="nmu")
                    musum = lnp.tile([128, 1], F32, tag="musum")
                    nc.vector.reduce_sum(musum[:], xt[:], axis=mybir.AxisListType.X)
                    nc.vector.tensor_scalar_mul(nmu[:], musum[:], -1.0 / DIM)
                    sq_dump = lnp.tile([128, DIM], BF16, tag="sqd")
                    sumsq = lnp.tile([128, 1], F32, tag="sumsq")
                    nc.scalar.activation(sq_dump[:], xt[:], AF.Square,
                                         bias=nmu[:], scale=1.0,
                                         accum_out=sumsq[:])
                    vareps = lnp.tile([128, 1], F32, tag="vareps")
                    nc.vector.tensor_scalar(vareps[:], sumsq[:], 1.0 / DIM, EPS,
                                            op0=ALU.mult, op1=ALU.add)
                    std = lnp.tile([128, 1], F32, tag="std")
                    nc.scalar.activation(std[:], vareps[:], AF.Sqrt)
                    rstd = lnp.tile([128, 1], F32, tag="rstd")
                    nc.vector.reciprocal(rstd[:], std[:])
                    nmr = lnp.tile([128, 1], F32, tag="nmr")
                    nc.vector.scalar_tensor_tensor(
                        nmr[:], nmu[:], 1.0, rstd[:],
                        op0=ALU.mult, op1=ALU.mult)
                    nc.scalar.activation(xn_sb[:, DIM * t:DIM * (t + 1)], xt[:],
                                         AF.Identity, bias=nmr[:], scale=rstd[:])

            # ================= P2: transpose xn -> xnT =====================
            xnT_sb = pers.tile([128, KT8 * TPC], BF16)  # [dim-tile partition, k*512+t128]
            with tc.tile_pool(name="ps_tr", bufs=6, space="PSUM") as pstr:
                for t in range(4):
                    for k in range(KT8):
                        trp = pstr.tile([128, 128], BF16, tag="tr")
                        nc.tensor.transpose(
                            trp[:], xn_sb[:, DIM * t + 128 * k: DIM * t + 128 * (k + 1)],
                            idn_sb[:])
                        nc.vector.tensor_copy(
                            xnT_sb[:, TPC * k + 128 * t: TPC * k + 128 * (t + 1)],
                            trp[:])

            # ---------------- other weight loads (background) -------------
            bias_sb = pers.tile([128, GT], F32)
            nc.sync.dma_start(bias_sb[:], bias_in[:])
            pwt_sb = pers.tile([128, KT8 * DIM], BF16)      # k-major proj weights
            nc.sync.dma_start(
                pwt_sb[:].rearrange("p (k o) -> p k o", o=DIM),
                pwt_in[:].rearrange("(k p) o -> p k o", p=128),
            )
            pbf_sb = pers.tile([128, DIM], BF16)
            nc.sync.dma_start(pbf_sb[:], pbf_in[:])
            tri_sb = pers.tile([128, 512], BF16)
            nc.sync.dma_start(tri_sb[:], tri_in[:])
            emat_sb = pers.tile([33, 128], BF16)
            nc.sync.dma_start(emat_sb[:], emat_in[:])
            sums_col = pers.tile([33, 512], F32)
            nc.vector.memset(sums_col[:], 1.0)

            # ================= P3: local token-parallel QKV ================
            # All 3072 qkv rows for this core's 512 tokens; rows are ordered
            # destination-core-major on the host, so row block 128*gt is the
            # (gt%3)-th third of chunk r=gt//3 of the AllToAll input.
            qkvL = pers.tile([128, GT * TPC], BF16)
            with tc.tile_pool(name="ps_q", bufs=3, space="PSUM") as psq:
                for gt in range(GT):
                    psg = psq.tile([128, TPC], F32, tag="q")
                    for k in range(KT8):
                        nc.tensor.matmul(
                            psg[:],
                            wt_sb[:, DIM * gt + 128 * k: DIM * gt + 128 * (k + 1)],
                            xnT_sb[:, TPC * k:TPC * (k + 1)],
                            start=(k == 0), stop=(k == KT8 - 1))
                    nc.vector.tensor_scalar(
                        qkvL[:, TPC * gt:TPC * (gt + 1)], psg[:],
                        bias_sb[:, gt:gt + 1], None, op0=ALU.add)
                    nc.sync.dma_start(qa_in[128 * gt:128 * (gt + 1), :],
                                      qkvL[:, TPC * gt:TPC * (gt + 1)])

            # ================= P4: AllToAll qkv ============================
            nc.gpsimd.collective_compute(
                "AllToAll", ALU.bypass, replica_groups=RG,
                ins=[qa_in[:].opt()], outs=[qa_out[:].opt()],
            )

            # -------- HAM keep-warm: dummy matmuls while the A2A is in flight
            with tc.tile_pool(name="ps_dummy", bufs=1, space="PSUM") as psd:
                dps = psd.tile([128, 512], F32, tag="d")
                for i in range(DUMMY_QA2A):
                    nc.tensor.matmul(dps[:], idn_sb[:],
                                     xnT_sb[:, 512 * (i % 8):512 * (i % 8) + 512],
                                     start=True, stop=True)

            # ================= P5/P6: head-parallel attention ==============
            qkvT = []
            for name in ("qT", "kT", "vT"):
                t_ = pers.tile([128, TOK], BF16, name=name)
                qkvT.append(t_)
            qT_sb, kT_sb, vT_sb = qkvT
            vnat = []
            for b in range(B):
                vb = pers.tile([128, 16 * 130], BF16, name=f"vnat{b}")
                nc.vector.memset(
                    vb[:].rearrange("p (j a w) -> p j a w", a=2, w=65)[:, :, :, 64:65], 1.0)
                vnat.append(vb)
            attnT = pers.tile([128, TOK], BF16)

            with (
                tc.tile_pool(name="pt", bufs=3) as ptp,
                tc.tile_pool(name="ps_s", bufs=3, space="PSUM") as pss,
                tc.tile_pool(name="ps_pv", bufs=4, space="PSUM") as psp,
                tc.tile_pool(name="ps_bc", bufs=1, space="PSUM") as psb,
                tc.tile_pool(name="sm", bufs=2) as smp,
            ):
                def emit_qload(s):
                    # my-heads q/k/v for source-core s's 512 tokens
                    for dst, off in ((qT_sb, 0), (kT_sb, 128), (vT_sb, 256)):
                        nc.sync.dma_start(
                            dst[:, TPC * s:TPC * (s + 1)],
                            qa_out[384 * s + off: 384 * s + off + 128, :])

                def emit_vtr(b, j):
                    vtr = psb.tile([128, 128], BF16, tag="bc")
                    nc.tensor.transpose(
                        vtr[:],
                        vT_sb[:, b * T + 128 * j: b * T + 128 * (j + 1)],
                        idn_sb[:])
                    nc.vector.tensor_copy(
                        vnat[b][:, 130 * j: 130 * j + 64], vtr[:, 0:64])
                    nc.vector.tensor_copy(
                        vnat[b][:, 130 * j + 65: 130 * j + 129], vtr[:, 64:128])

                def emit_attention(b, inject):
                    ii = 0
                    for qc in range(4):
                        q0 = b * T + 512 * qc
                        r = 4 * b + qc
                        pvA = psp.tile([65, 512], F32, tag="pv")
                        pvB = psp.tile([65, 512], F32, tag="pv")
                        nkp = 4 * qc + 4
                        pend = None
                        for kp in range(nkp):
                            k0 = b * T + 128 * kp
                            j = kp - 4 * qc       # >=0 -> diagonal block
                            o = 128 * j if j > 0 else 0
                            n = 512 - o
                            sA = pss.tile([128, 512], F32, tag="s")
                            sB = pss.tile([128, 512], F32, tag="s")
                            nc.tensor.matmul(sA[:, o:512], kT_sb[0:64, k0:k0 + 128],
                                             qT_sb[0:64, q0 + o:q0 + 512],
                                             start=True, stop=True)
                            nc.tensor.matmul(sB[:, o:512], kT_sb[64:128, k0:k0 + 128],
                                             qT_sb[64:128, q0 + o:q0 + 512],
                                             start=True, stop=True)
                            if pend is not None:
                                pkp, ppA, ppB, po = pend
                                nc.tensor.matmul(pvA[:, po:512],
                                                 vnat[b][:, 130 * pkp:130 * pkp + 65],
                                                 ppA[:, po:512],
                                                 start=(pkp == 0), stop=False,
                                                 skip_group_check=True)
                                nc.tensor.matmul(pvB[:, po:512],
                                                 vnat[b][:, 130 * pkp + 65:130 * pkp + 130],
                                                 ppB[:, po:512],
                                                 start=(pkp == 0), stop=False,
                                                 skip_group_check=True)
                            pA = ptp.tile([128, 512], BF16, tag="pA")
                            pB = ptp.tile([128, 512], BF16, tag="pB")
                            nc.scalar.activation(pA[:, o:512], sA[:, o:512], AF.Exp)
                            nc.scalar.activation(pB[:, o:512], sB[:, o:512], AF.Exp)
                            if j >= 0:
                                nc.vector.tensor_tensor(
                                    pA[:, o:512], pA[:, o:512], tri_sb[:, 0:n],
                                    op=ALU.mult)
                                nc.gpsimd.tensor_tensor(
                                    pB[:, o:512], pB[:, o:512], tri_sb[:, 0:n],
                                    op=ALU.mult)
                            pend = (kp, pA, pB, o)
                            if ii < len(inject):
                                inject[ii]()
                                ii += 1
                        pkp, ppA, ppB, po = pend
                        nc.tensor.matmul(pvA[:, po:512],
                                         vnat[b][:, 130 * pkp:130 * pkp + 65],
                                         ppA[:, po:512],
                                         start=(pkp == 0), stop=True,
                                         skip_group_check=True)
                        nc.tensor.matmul(pvB[:, po:512],
                                         vnat[b][:, 130 * pkp + 65:130 * pkp + 130],
                                         ppB[:, po:512],
                                         start=(pkp == 0), stop=True,
                                         skip_group_check=True)
                        nc.vector.tensor_copy(sums_col[0:1, :], pvA[64:65, :])
                        nc.vector.tensor_copy(sums_col[32:33, :], pvB[64:65, :])
                        rec = smp.tile([33, 512], F32, tag="rec")
                        nc.vector.reciprocal_approx_fast(rec[:], sums_col[:])
                        recb = smp.tile([33, 512], BF16, tag="recb")
                        nc.vector.tensor_copy(recb[:], rec[:])
                        bc2 = psb.tile([128, 512], F32, tag="bc")
                        nc.tensor.matmul(bc2[:], emat_sb[:], recb[:],
                                         start=True, stop=True)
                        bc2s = smp.tile([128, 512], BF16, tag="bc2s")
                        nc.vector.tensor_copy(bc2s[:], bc2[:])
                        nc.vector.tensor_tensor(
                            attnT[0:64, q0:q0 + 512], pvA[0:64, :],
                            bc2s[0:64, :], op=ALU.mult)
                        nc.vector.tensor_tensor(
                            attnT[64:128, q0:q0 + 512], pvB[0:64, :],
                            bc2s[64:128, :], op=ALU.mult)
                        nc.sync.dma_start(ao_in[128 * r:128 * (r + 1), :],
                                          attnT[:, TPC * r:TPC * (r + 1)])
                    while ii < len(inject):
                        inject[ii]()
                        ii += 1

                emit_qload(0)
                for j in range(4):
                    emit_vtr(0, j)
                inject_b0 = [
                    lambda: emit_qload(1),
                    lambda: emit_vtr(0, 4),
                    lambda: emit_vtr(0, 5),
                    lambda: emit_vtr(0, 6),
                    lambda: emit_vtr(0, 7),
                    lambda: emit_qload(2),
                    lambda: emit_vtr(0, 8),
                    lambda: emit_vtr(0, 9),
                    lambda: emit_vtr(0, 10),
                    lambda: emit_vtr(0, 11),
                    lambda: emit_qload(3),
                    lambda: emit_vtr(0, 12),
                    lambda: emit_vtr(0, 13),
                    lambda: emit_vtr(0, 14),
                    lambda: emit_vtr(0, 15),
                    lambda: emit_qload(4),
                    lambda: emit_qload(5),
                    lambda: emit_qload(6),
                    lambda: emit_qload(7),
                ] + [
                    (lambda jj=j: emit_vtr(1, jj)) for j in range(16)
                ]
                emit_attention(0, inject_b0)
                emit_attention(1, [])

            # ================= P7: AllToAll attention outputs ==============
            # (per-qc staging DMAs already issued inside emit_attention)
            nc.gpsimd.collective_compute(
                "AllToAll", ALU.bypass, replica_groups=RG,
                ins=[ao_in[:].opt()], outs=[ao_out[:].opt()],
            )

            # -------- HAM keep-warm: dummy matmuls while the A2A is in flight
            with tc.tile_pool(name="ps_dummy2", bufs=1, space="PSUM") as psd2:
                dps2 = psd2.tile([128, 512], F32, tag="d2")
                for i in range(DUMMY_A2A):
                    nc.tensor.matmul(dps2[:], idn_sb[:],
                                     attnT[:, 512 * (i % 8):512 * (i % 8) + 512],
                                     start=True, stop=True)

            # ================= P8: output projection (token slice) =========
            with (
                tc.tile_pool(name="projx", bufs=1) as pxp,
                tc.tile_pool(name="ps_o", bufs=4, space="PSUM") as pso,
                tc.tile_pool(name="outp", bufs=2) as outp,
            ):
                aT = []
                for ck in range(KT8):
                    ak = pxp.tile([128, TPC], BF16, tag=f"aT{ck}", name=f"ak{ck}")
                    nc.sync.dma_start(ak[:],
                                      ao_out[128 * ck:128 * (ck + 1), :])
                    aT.append(ak)
                for tt in range(4):
                    ps0 = pso.tile([128, 512], F32, tag="po")
                    ps1 = pso.tile([128, 512], F32, tag="po")
                    for ck in range(KT8):
                        lh = aT[ck][:, 128 * tt: 128 * (tt + 1)]
                        nc.tensor.matmul(ps0[:], lh,
                                         pwt_sb[:, DIM * ck: DIM * ck + 512],
                                         start=(ck == 0), stop=(ck == KT8 - 1))
                        nc.tensor.matmul(ps1[:], lh,
                                         pwt_sb[:, DIM * ck + 512: DIM * ck + 1024],
                                         start=(ck == 0), stop=(ck == KT8 - 1))
                    ot = outp.tile([128, DIM], F32, tag="ot")
                    nc.vector.tensor_tensor(ot[:, 0:512], ps0[:],
                                            pbf_sb[:, 0:512], op=ALU.add)
                    nc.vector.tensor_tensor(ot[:, 512:1024], ps1[:],
                                            pbf_sb[:, 512:1024], op=ALU.add)
                    nc.sync.dma_start(out_dram[128 * tt:128 * (tt + 1), :], ot[:])

    nc.compile()
    return nc


def host_prep(inputs):
    x = np.asarray(inputs["x"], np.float32).reshape(TOK, DIM)
    ln_w = np.asarray(inputs["ln_w"], np.float32)
    ln_b = np.asarray(inputs["ln_b"], np.float32)
    qkv_w = np.asarray(inputs["qkv_w"], np.float32)
    qkv_b = np.asarray(inputs["qkv_b"], np.float32)
    proj_w = np.asarray(inputs["proj_w"], np.float32)
    proj_b = np.asarray(inputs["proj_b"], np.float32)

    # fold LN affine into qkv weights; fold 1/sqrt(D) into Q rows
    Wp = qkv_w * ln_w[None, :]
    bp = qkv_b + qkv_w @ ln_b
    Wp[0:DIM] *= D ** -0.5
    bp[0:DIM] *= D ** -0.5

    # destination-core-major row permutation: for core c, its 384 rows are
    # [q(h2c), q(h2c+1), k(h2c), k(h2c+1), v(h2c), v(h2c+1)]
    rows = []
    for c in range(NC):
        for blk in range(3):
            for h in (2 * c, 2 * c + 1):
                rows.extend(range(blk * DIM + h * D, blk * DIM + (h + 1) * D))
    rows = np.array(rows)
    Wperm = Wp[rows]                      # [3072, 1024]
    bperm = bp[rows]                      # [3072]

    idn = np.eye(128, dtype=np.float32).astype(BF16_NP)
    tri = (np.arange(512)[None, :] >= np.arange(128)[:, None]).astype(BF16_NP)
    emat = np.zeros((33, 128), np.float32)
    emat[0, 0:64] = 1.0
    emat[32, 64:128] = 1.0
    emat = emat.astype(BF16_NP)
    pwt = proj_w.T.copy().astype(BF16_NP)
    pbf = np.broadcast_to(proj_b.reshape(1, DIM), (128, DIM)).copy().astype(BF16_NP)
    wt_c = np.ascontiguousarray(Wperm.T).astype(BF16_NP)       # [1024, 3072]
    bias_c = np.ascontiguousarray(bperm.reshape(GT, 128).T)    # [128, 24]

    in_maps = []
    for c in range(NC):
        in_maps.append(dict(
            x_c=np.ascontiguousarray(x[TPC * c:TPC * (c + 1)]),
            wt_c=wt_c, bias_c=bias_c,
            pwt=pwt, pbf=pbf, idn=idn, tri=tri, emat=emat,
        ))
    return in_maps


_CACHED = {}


def kernel(**inputs) -> np.ndarray:
    _ensure_ntff_hook()
    from concourse import bass_utils
    if TRACE:
        bass_utils.upload_artifacts = lambda tmpdir: "/tmp/noupload"

    if "nc" not in _CACHED:
        _CACHED["nc"] = build_graph()
    nc = _CACHED["nc"]

    in_maps = host_prep(inputs)
    res = bass_utils.run_bass_kernel_spmd(
        nc, in_maps, core_ids=list(range(NC)), trace=TRACE,
        trace_cores=list(range(NC)) if TRACE else None)
    _CACHED["last_result"] = res
    out = np.concatenate([res.results[c]["out_c"] for c in range(NC)], axis=0)
    return out.reshape(B, T, DIM).astype(np.float32)


# revision 36
# speedup vs baseline: 1.0734x; 1.0734x over previous
"""Distributed Trainium2 Bass kernel for fused LayerNorm + causal multi-head
attention + output projection (B=2, T=2048, DIM=1024, H=16, D=64) on 8 cores.

Sharding (v6):
  - LayerNorm + QKV projection + final projection: token-parallel
    (512 tokens/core). QKV is computed on LOCAL data (full 3072-row weight)
    BEFORE any collective, so the first-collective rendezvous (launch skew)
    is absorbed by ~60us of real matmul work instead of idle waiting.
  - qkv travels via one bf16 AllToAll into head-parallel layout
    (2 heads x 2 batches per core); attention outputs return via a second
    bf16 AllToAll; projection is token-parallel again.
  - causal diagonal blocks are N-trimmed; triangular mask via precomputed
    bf16 multiply (DVE+Pool); denominators via the vnat ones-column trick.

Compute dtype: bf16 matmuls with fp32 PSUM accumulation (rel err ~5e-3).
LN affine params and the 1/sqrt(D) score scale are folded into the QKV
weights on the host.
"""
import os
import sys
import types
import numpy as np
import ml_dtypes

# ---------------------------------------------------------------- constants
B, T, DIM, D = 2, 2048, 1024, 64
H = DIM // D            # 16 heads
NC = 8                  # cores
TOK = B * T             # 4096 tokens
TPC = TOK // NC         # 512 tokens per core
KT8 = DIM // 128        # 8 contraction tiles
GT = 3 * DIM // 128     # 24 qkv output tiles of 128 rows
EPS = 1e-5

TRACE = bool(int(os.environ.get("BASS_KERNEL_TRACE", "0")))
DUMMY_QA2A = int(os.environ.get("DUMMY_QA2A", "110"))
DUMMY_A2A = int(os.environ.get("DUMMY_A2A", "85"))

BF16_NP = ml_dtypes.bfloat16


def _ensure_ntff_hook():
    """The agent image lacks antenv.axon_hooks; recreate it so trace=True works."""
    if "antenv.axon_hooks" not in sys.modules:
        mod = types.ModuleType("antenv.axon_hooks")
        mod._hook = None
        def set_axon_ntff_profile_hook(h):
            mod._hook = h
        def get_axon_ntff_profile_hook():
            return mod._hook
        mod.set_axon_ntff_profile_hook = set_axon_ntff_profile_hook
        mod.get_axon_ntff_profile_hook = get_axon_ntff_profile_hook
        sys.modules["antenv.axon_hooks"] = mod
    m = sys.modules["antenv.axon_hooks"]
    if m.get_axon_ntff_profile_hook() is None:
        try:
            from trn_agent_boot.trn_boot import _ntff_profile_via_ctypes
            m.set_axon_ntff_profile_hook(
                _ntff_profile_via_ctypes("/opt/axon/libaxon_pjrt.so"))
        except Exception:
            pass


def build_graph():
    import concourse.bass as bass
    import concourse.bacc as bacc
    import concourse.tile as tile
    import concourse.mybir as mybir

    dt = mybir.dt
    F32, BF16 = dt.float32, dt.bfloat16
    AF = mybir.ActivationFunctionType
    ALU = mybir.AluOpType
    RG = [list(range(NC))]

    nc = bacc.Bacc(None, target_bir_lowering=False, debug=False, num_devices=NC)

    # ------------------------------------------------------------ I/O
    x_in = nc.dram_tensor("x_c", [TPC, DIM], F32, kind="ExternalInput")
    wt_in = nc.dram_tensor("wt_c", [DIM, 3 * DIM], BF16, kind="ExternalInput")
    bias_in = nc.dram_tensor("bias_c", [128, GT], F32, kind="ExternalInput")
    pwt_in = nc.dram_tensor("pwt", [DIM, DIM], BF16, kind="ExternalInput")
    pbf_in = nc.dram_tensor("pbf", [128, DIM], BF16, kind="ExternalInput")
    idn_in = nc.dram_tensor("idn", [128, 128], BF16, kind="ExternalInput")
    tri_in = nc.dram_tensor("tri", [128, 512], BF16, kind="ExternalInput")
    emat_in = nc.dram_tensor("emat", [33, 128], BF16, kind="ExternalInput")
    out_dram = nc.dram_tensor("out_c", [TPC, DIM], F32, kind="ExternalOutput")

    with tile.TileContext(nc) as tc:
        with (
            tc.tile_pool(name="persist", bufs=1) as pers,
            tc.tile_pool(name="dram", bufs=1, space="DRAM") as dram,
        ):
            # ---------------- DRAM bounce buffers ----------------
            qa1_in = dram.tile([NC * 256, TPC], BF16)         # q+k AllToAll
            qa1_out = dram.tile([NC * 256, TPC], BF16)
            qa2_in = dram.tile([NC * 128, TPC], BF16)         # v AllToAll
            qa2_out = dram.tile([NC * 128, TPC], BF16)
            ao_in = dram.tile([NC * 128, TPC], BF16)          # attn-out AllToAll
            ao_out = dram.tile([NC * 128, TPC], BF16)

            # idn first: transposes need it early; it is tiny
            idn_sb = pers.tile([128, 128], BF16)
            nc.sync.dma_start(idn_sb[:], idn_in[:])

            # ================= P1: LayerNorm (token slice, natural) ========
            xn_sb = pers.tile([128, 4 * DIM], BF16)   # 4 token tiles side by side
            wt_sb = pers.tile([128, GT * DIM], BF16)  # gt-major, k-minor qkv weights
            with tc.tile_pool(name="ln", bufs=4) as lnp:
                # x tiles first on the DMA queue, then the 24 weight-tile DMAs
                xts = []
                for t in range(4):
                    xt = lnp.tile([128, DIM], F32, tag="xt", name=f"xt{t}")
                    nc.sync.dma_start(xt[:], x_in[128 * t:128 * (t + 1), :])
                    xts.append(xt)
                qk_gts = [gt for gt in range(GT) if gt % 3 != 2]
                v_gts = [gt for gt in range(GT) if gt % 3 == 2]
                for gt in qk_gts + v_gts:
                    nc.sync.dma_start(
                        wt_sb[:, DIM * gt:DIM * (gt + 1)]
                        .rearrange("p (k o) -> p k o", o=128),
                        wt_in[:, 128 * gt:128 * (gt + 1)]
                        .rearrange("(k p) o -> p k o", p=128),
                    )
                for t in range(4):
                    xt = xts[t]
                    nmu = lnp.tile([128, 1], F32, tag="nmu")
                    musum = lnp.tile([128, 1], F32, tag="musum")
                    nc.vector.reduce_sum(musum[:], xt[:], axis=mybir.AxisListType.X)
                    nc.vector.tensor_scalar_mul(nmu[:], musum[:], -1.0 / DIM)
                    sq_dump = lnp.tile([128, DIM], BF16, tag="sqd")
                    sumsq = lnp.tile([128, 1], F32, tag="sumsq")
                    nc.scalar.activation(sq_dump[:], xt[:], AF.Square,
                                         bias=nmu[:], scale=1.0,
                                         accum_out=sumsq[:])
                    vareps = lnp.tile([128, 1], F32, tag="vareps")
                    nc.vector.tensor_scalar(vareps[:], sumsq[:], 1.0 / DIM, EPS,
                                            op0=ALU.mult, op1=ALU.add)
                    std = lnp.tile([128, 1], F32, tag="std")
                    nc.scalar.activation(std[:], vareps[:], AF.Sqrt)
                    rstd = lnp.tile([128, 1], F32, tag="rstd")
                    nc.vector.reciprocal(rstd[:], std[:])
                    nmr = lnp.tile([128, 1], F32, tag="nmr")
                    nc.vector.scalar_tensor_tensor(
                        nmr[:], nmu[:], 1.0, rstd[:],
                        op0=ALU.mult, op1=ALU.mult)
                    nc.scalar.activation(xn_sb[:, DIM * t:DIM * (t + 1)], xt[:],
                                         AF.Identity, bias=nmr[:], scale=rstd[:])

            # ================= P2: transpose xn -> xnT =====================
            xnT_sb = pers.tile([128, KT8 * TPC], BF16)  # [dim-tile partition, k*512+t128]
            with tc.tile_pool(name="ps_tr", bufs=6, space="PSUM") as pstr:
                for t in range(4):
                    for k in range(KT8):
                        trp = pstr.tile([128, 128], BF16, tag="tr")
                        nc.tensor.transpose(
                            trp[:], xn_sb[:, DIM * t + 128 * k: DIM * t + 128 * (k + 1)],
                            idn_sb[:])
                        nc.vector.tensor_copy(
                            xnT_sb[:, TPC * k + 128 * t: TPC * k + 128 * (t + 1)],
                            trp[:])

            # ---------------- other weight loads (background) -------------
            bias_sb = pers.tile([128, GT], F32)
            nc.sync.dma_start(bias_sb[:], bias_in[:])
            pwt_sb = pers.tile([128, KT8 * DIM], BF16)      # k-major proj weights
            nc.sync.dma_start(
                pwt_sb[:].rearrange("p (k o) -> p k o", o=DIM),
                pwt_in[:].rearrange("(k p) o -> p k o", p=128),
            )
            pbf_sb = pers.tile([128, DIM], BF16)
            nc.sync.dma_start(pbf_sb[:], pbf_in[:])
            tri_sb = pers.tile([128, 512], BF16)
            nc.sync.dma_start(tri_sb[:], tri_in[:])
            emat_sb = pers.tile([33, 128], BF16)
            nc.sync.dma_start(emat_sb[:], emat_in[:])
            sums_col = pers.tile([33, 512], F32)
            nc.vector.memset(sums_col[:], 1.0)

            # ================= P3: local token-parallel QKV ================
            # All 3072 qkv rows for this core's 512 tokens; rows are ordered
            # destination-core-major on the host, so row block 128*gt is the
            # (gt%3)-th third of chunk r=gt//3 of the AllToAll input.
            qkvL = pers.tile([128, GT * TPC], BF16)
            with tc.tile_pool(name="ps_q", bufs=3, space="PSUM") as psq:
                def emit_qkv(gt):
                    psg = psq.tile([128, TPC], F32, tag="q")
                    for k in range(KT8):
                        nc.tensor.matmul(
                            psg[:],
                            wt_sb[:, DIM * gt + 128 * k: DIM * gt + 128 * (k + 1)],
                            xnT_sb[:, TPC * k:TPC * (k + 1)],
                            start=(k == 0), stop=(k == KT8 - 1))
                    nc.vector.tensor_scalar(
                        qkvL[:, TPC * gt:TPC * (gt + 1)], psg[:],
                        bias_sb[:, gt:gt + 1], None, op0=ALU.add)
                    r, c = gt // 3, gt % 3
                    if c == 2:
                        dst = qa2_in[128 * r:128 * (r + 1), :]
                    else:
                        dst = qa1_in[256 * r + 128 * c: 256 * r + 128 * (c + 1), :]
                    nc.sync.dma_start(dst, qkvL[:, TPC * gt:TPC * (gt + 1)])

                # q+k tiles first -> AllToAll #1 overlaps the v tiles' matmuls
                for gt in qk_gts:
                    emit_qkv(gt)
                nc.gpsimd.collective_compute(
                    "AllToAll", ALU.bypass, replica_groups=RG,
                    ins=[qa1_in[:].opt()], outs=[qa1_out[:].opt()],
                )
                for gt in v_gts:
                    emit_qkv(gt)
                nc.gpsimd.collective_compute(
                    "AllToAll", ALU.bypass, replica_groups=RG,
                    ins=[qa2_in[:].opt()], outs=[qa2_out[:].opt()],
                )

            # -------- HAM keep-warm: dummy matmuls while the A2As are in flight
            with tc.tile_pool(name="ps_dummy", bufs=1, space="PSUM") as psd:
                dps = psd.tile([128, 512], F32, tag="d")
                for i in range(DUMMY_QA2A):
                    nc.tensor.matmul(dps[:], idn_sb[:],
                                     xnT_sb[:, 512 * (i % 8):512 * (i % 8) + 512],
                                     start=True, stop=True)

            # ================= P5/P6: head-parallel attention ==============
            qkvT = []
            for name in ("qT", "kT", "vT"):
                t_ = pers.tile([128, TOK], BF16, name=name)
                qkvT.append(t_)
            qT_sb, kT_sb, vT_sb = qkvT
            vnat = []
            for b in range(B):
                vb = pers.tile([128, 16 * 130], BF16, name=f"vnat{b}")
                nc.vector.memset(
                    vb[:].rearrange("p (j a w) -> p j a w", a=2, w=65)[:, :, :, 64:65], 1.0)
                vnat.append(vb)
            attnT = pers.tile([128, TOK], BF16)

            with (
                tc.tile_pool(name="pt", bufs=3) as ptp,
                tc.tile_pool(name="ps_s", bufs=3, space="PSUM") as pss,
                tc.tile_pool(name="ps_pv", bufs=4, space="PSUM") as psp,
                tc.tile_pool(name="ps_bc", bufs=1, space="PSUM") as psb,
                tc.tile_pool(name="sm", bufs=2) as smp,
            ):
                def emit_qload(s):
                    # my-heads q/k/v for source-core s's 512 tokens
                    nc.sync.dma_start(qT_sb[:, TPC * s:TPC * (s + 1)],
                                      qa1_out[256 * s: 256 * s + 128, :])
                    nc.sync.dma_start(kT_sb[:, TPC * s:TPC * (s + 1)],
                                      qa1_out[256 * s + 128: 256 * s + 256, :])
                    nc.sync.dma_start(vT_sb[:, TPC * s:TPC * (s + 1)],
                                      qa2_out[128 * s: 128 * s + 128, :])

                def emit_vtr(b, j):
                    vtr = psb.tile([128, 128], BF16, tag="bc")
                    nc.tensor.transpose(
                        vtr[:],
                        vT_sb[:, b * T + 128 * j: b * T + 128 * (j + 1)],
                        idn_sb[:])
                    nc.vector.tensor_copy(
                        vnat[b][:, 130 * j: 130 * j + 64], vtr[:, 0:64])
                    nc.vector.tensor_copy(
                        vnat[b][:, 130 * j + 65: 130 * j + 129], vtr[:, 64:128])

                def emit_attention(b, inject):
                    ii = 0
                    for qc in range(4):
                        q0 = b * T + 512 * qc
                        r = 4 * b + qc
                        pvA = psp.tile([65, 512], F32, tag="pv")
                        pvB = psp.tile([65, 512], F32, tag="pv")
                        nkp = 4 * qc + 4
                        pend = None
                        for kp in range(nkp):
                            k0 = b * T + 128 * kp
                            j = kp - 4 * qc       # >=0 -> diagonal block
                            o = 128 * j if j > 0 else 0
                            n = 512 - o
                            sA = pss.tile([128, 512], F32, tag="s")
                            sB = pss.tile([128, 512], F32, tag="s")
                            nc.tensor.matmul(sA[:, o:512], kT_sb[0:64, k0:k0 + 128],
                                             qT_sb[0:64, q0 + o:q0 + 512],
                                             start=True, stop=True)
                            nc.tensor.matmul(sB[:, o:512], kT_sb[64:128, k0:k0 + 128],
                                             qT_sb[64:128, q0 + o:q0 + 512],
                                             start=True, stop=True)
                            if pend is not None:
                                pkp, ppA, ppB, po = pend
                                nc.tensor.matmul(pvA[:, po:512],
                                                 vnat[b][:, 130 * pkp:130 * pkp + 65],
                                                 ppA[:, po:512],
                                                 start=(pkp == 0), stop=False,
                                                 skip_group_check=True)
                                nc.tensor.matmul(pvB[:, po:512],
                                                 vnat[b][:, 130 * pkp + 65:130 * pkp + 130],
                                                 ppB[:, po:512],
                                                 start=(pkp == 0), stop=False,
                                                 skip_group_check=True)
                            pA = ptp.tile([128, 512], BF16, tag="pA")
                            pB = ptp.tile([128, 512], BF16, tag="pB")
                            nc.scalar.activation(pA[:, o:512], sA[:, o:512], AF.Exp)
                            nc.scalar.activation(pB[:, o:512], sB[:, o:512], AF.Exp)
                            if j >= 0:
                                nc.vector.tensor_tensor(
                                    pA[:, o:512], pA[:, o:512], tri_sb[:, 0:n],
                                    op=ALU.mult)
                                nc.gpsimd.tensor_tensor(
                                    pB[:, o:512], pB[:, o:512], tri_sb[:, 0:n],
                                    op=ALU.mult)
                            pend = (kp, pA, pB, o)
                            if ii < len(inject):
                                inject[ii]()
                                ii += 1
                        pkp, ppA, ppB, po = pend
                        nc.tensor.matmul(pvA[:, po:512],
                                         vnat[b][:, 130 * pkp:130 * pkp + 65],
                                         ppA[:, po:512],
                                         start=(pkp == 0), stop=True,
                                         skip_group_check=True)
                        nc.tensor.matmul(pvB[:, po:512],
                                         vnat[b][:, 130 * pkp + 65:130 * pkp + 130],
                                         ppB[:, po:512],
                                         start=(pkp == 0), stop=True,
                                         skip_group_check=True)
                        nc.vector.tensor_copy(sums_col[0:1, :], pvA[64:65, :])
                        nc.vector.tensor_copy(sums_col[32:33, :], pvB[64:65, :])
                        rec = smp.tile([33, 512], F32, tag="rec")
                        nc.vector.reciprocal_approx_fast(rec[:], sums_col[:])
                        recb = smp.tile([33, 512], BF16, tag="recb")
                        nc.vector.tensor_copy(recb[:], rec[:])
                        bc2 = psb.tile([128, 512], F32, tag="bc")
                        nc.tensor.matmul(bc2[:], emat_sb[:], recb[:],
                                         start=True, stop=True)
                        bc2s = smp.tile([128, 512], BF16, tag="bc2s")
                        nc.vector.tensor_copy(bc2s[:], bc2[:])
                        nc.vector.tensor_tensor(
                            attnT[0:64, q0:q0 + 512], pvA[0:64, :],
                            bc2s[0:64, :], op=ALU.mult)
                        nc.vector.tensor_tensor(
                            attnT[64:128, q0:q0 + 512], pvB[0:64, :],
                            bc2s[64:128, :], op=ALU.mult)
                        nc.sync.dma_start(ao_in[128 * r:128 * (r + 1), :],
                                          attnT[:, TPC * r:TPC * (r + 1)])
                    while ii < len(inject):
                        inject[ii]()
                        ii += 1

                emit_qload(0)
                for j in range(4):
                    emit_vtr(0, j)
                inject_b0 = [
                    lambda: emit_qload(1),
                    lambda: emit_vtr(0, 4),
                    lambda: emit_vtr(0, 5),
                    lambda: emit_vtr(0, 6),
                    lambda: emit_vtr(0, 7),
                    lambda: emit_qload(2),
                    lambda: emit_vtr(0, 8),
                    lambda: emit_vtr(0, 9),
                    lambda: emit_vtr(0, 10),
                    lambda: emit_vtr(0, 11),
                    lambda: emit_qload(3),
                    lambda: emit_vtr(0, 12),
                    lambda: emit_vtr(0, 13),
                    lambda: emit_vtr(0, 14),
                    lambda: emit_vtr(0, 15),
                    lambda: emit_qload(4),
                    lambda: emit_qload(5),
                    lambda: emit_qload(6),
                    lambda: emit_qload(7),
                ] + [
                    (lambda jj=j: emit_vtr(1, jj)) for j in range(16)
                ]
                emit_attention(0, inject_b0)
                emit_attention(1, [])

            # ================= P7: AllToAll attention outputs ==============
            # (per-qc staging DMAs already issued inside emit_attention)
            nc.gpsimd.collective_compute(
                "AllToAll", ALU.bypass, replica_groups=RG,
                ins=[ao_in[:].opt()], outs=[ao_out[:].opt()],
            )

            # -------- HAM keep-warm: dummy matmuls while the A2A is in flight
            with tc.tile_pool(name="ps_dummy2", bufs=1, space="PSUM") as psd2:
                dps2 = psd2.tile([128, 512], F32, tag="d2")
                for i in range(DUMMY_A2A):
                    nc.tensor.matmul(dps2[:], idn_sb[:],
                                     attnT[:, 512 * (i % 8):512 * (i % 8) + 512],
                                     start=True, stop=True)

            # ================= P8: output projection (token slice) =========
            with (
                tc.tile_pool(name="projx", bufs=1) as pxp,
                tc.tile_pool(name="ps_o", bufs=4, space="PSUM") as pso,
                tc.tile_pool(name="outp", bufs=2) as outp,
            ):
                aT = []
                for ck in range(KT8):
                    ak = pxp.tile([128, TPC], BF16, tag=f"aT{ck}", name=f"ak{ck}")
                    nc.sync.dma_start(ak[:],
                                      ao_out[128 * ck:128 * (ck + 1), :])
                    aT.append(ak)
                for tt in range(4):
                    ps0 = pso.tile([128, 512], F32, tag="po")
                    ps1 = pso.tile([128, 512], F32, tag="po")
                    for ck in range(KT8):
                        lh = aT[ck][:, 128 * tt: 128 * (tt + 1)]
                        nc.tensor.matmul(ps0[:], lh,
                                         pwt_sb[:, DIM * ck: DIM * ck + 512],
                                         start=(ck == 0), stop=(ck == KT8 - 1))
                        nc.tensor.matmul(ps1[:], lh,
                                         pwt_sb[:, DIM * ck + 512: DIM * ck + 1024],
                                         start=(ck == 0), stop=(ck == KT8 - 1))
                    ot = outp.tile([128, DIM], F32, tag="ot")
                    nc.vector.tensor_tensor(ot[:, 0:512], ps0[:],
                                            pbf_sb[:, 0:512], op=ALU.add)
                    nc.vector.tensor_tensor(ot[:, 512:1024], ps1[:],
                                            pbf_sb[:, 512:1024], op=ALU.add)
                    nc.sync.dma_start(out_dram[128 * tt:128 * (tt + 1), :], ot[:])

    nc.compile()
    return nc


def host_prep(inputs):
    x = np.asarray(inputs["x"], np.float32).reshape(TOK, DIM)
    ln_w = np.asarray(inputs["ln_w"], np.float32)
    ln_b = np.asarray(inputs["ln_b"], np.float32)
    qkv_w = np.asarray(inputs["qkv_w"], np.float32)
    qkv_b = np.asarray(inputs["qkv_b"], np.float32)
    proj_w = np.asarray(inputs["proj_w"], np.float32)
    proj_b = np.asarray(inputs["proj_b"], np.float32)

    # fold LN affine into qkv weights; fold 1/sqrt(D) into Q rows
    Wp = qkv_w * ln_w[None, :]
    bp = qkv_b + qkv_w @ ln_b
    Wp[0:DIM] *= D ** -0.5
    bp[0:DIM] *= D ** -0.5

    # destination-core-major row permutation: for core c, its 384 rows are
    # [q(h2c), q(h2c+1), k(h2c), k(h2c+1), v(h2c), v(h2c+1)]
    rows = []
    for c in range(NC):
        for blk in range(3):
            for h in (2 * c, 2 * c + 1):
                rows.extend(range(blk * DIM + h * D, blk * DIM + (h + 1) * D))
    rows = np.array(rows)
    Wperm = Wp[rows]                      # [3072, 1024]
    bperm = bp[rows]                      # [3072]

    idn = np.eye(128, dtype=np.float32).astype(BF16_NP)
    tri = (np.arange(512)[None, :] >= np.arange(128)[:, None]).astype(BF16_NP)
    emat = np.zeros((33, 128), np.float32)
    emat[0, 0:64] = 1.0
    emat[32, 64:128] = 1.0
    emat = emat.astype(BF16_NP)
    pwt = proj_w.T.copy().astype(BF16_NP)
    pbf = np.broadcast_to(proj_b.reshape(1, DIM), (128, DIM)).copy().astype(BF16_NP)
    wt_c = np.ascontiguousarray(Wperm.T).astype(BF16_NP)       # [1024, 3072]
    bias_c = np.ascontiguousarray(bperm.reshape(GT, 128).T)    # [128, 24]

    in_maps = []
    for c in range(NC):
        in_maps.append(dict(
            x_c=np.ascontiguousarray(x[TPC * c:TPC * (c + 1)]),
            wt_c=wt_c, bias_c=bias_c,
            pwt=pwt, pbf=pbf, idn=idn, tri=tri, emat=emat,
        ))
    return in_maps


_CACHED = {}


def kernel(**inputs) -> np.ndarray:
    _ensure_ntff_hook()
    from concourse import bass_utils
    if TRACE:
        bass_utils.upload_artifacts = lambda tmpdir: "/tmp/noupload"

    if "nc" not in _CACHED:
        _CACHED["nc"] = build_graph()
    nc = _CACHED["nc"]

    in_maps = host_prep(inputs)
    res = bass_utils.run_bass_kernel_spmd(
        nc, in_maps, core_ids=list(range(NC)), trace=TRACE,
        trace_cores=list(range(NC)) if TRACE else None)
    _CACHED["last_result"] = res
    out = np.concatenate([res.results[c]["out_c"] for c in range(NC)], axis=0)
    return out.reshape(B, T, DIM).astype(np.float32)


# revision 40
# speedup vs baseline: 1.0909x; 1.0163x over previous
"""Distributed Trainium2 Bass kernel for fused LayerNorm + causal multi-head
attention + output projection (B=2, T=2048, DIM=1024, H=16, D=64) on 8 cores.

Sharding (v6):
  - LayerNorm + QKV projection + final projection: token-parallel
    (512 tokens/core). QKV is computed on LOCAL data (full 3072-row weight)
    BEFORE any collective, so the first-collective rendezvous (launch skew)
    is absorbed by ~60us of real matmul work instead of idle waiting.
  - qkv travels via one bf16 AllToAll into head-parallel layout
    (2 heads x 2 batches per core); attention outputs return via a second
    bf16 AllToAll; projection is token-parallel again.
  - causal diagonal blocks are N-trimmed; triangular mask via precomputed
    bf16 multiply (DVE+Pool); denominators via the vnat ones-column trick.

Compute dtype: bf16 matmuls with fp32 PSUM accumulation (rel err ~5e-3).
LN affine params and the 1/sqrt(D) score scale are folded into the QKV
weights on the host.
"""
import os
import sys
import types
import numpy as np
import ml_dtypes

# ---------------------------------------------------------------- constants
B, T, DIM, D = 2, 2048, 1024, 64
H = DIM // D            # 16 heads
NC = 8                  # cores
TOK = B * T             # 4096 tokens
TPC = TOK // NC         # 512 tokens per core
KT8 = DIM // 128        # 8 contraction tiles
GT = 3 * DIM // 128     # 24 qkv output tiles of 128 rows
EPS = 1e-5

TRACE = bool(int(os.environ.get("BASS_KERNEL_TRACE", "0")))
DUMMY_QA2A = int(os.environ.get("DUMMY_QA2A", "160"))
DUMMY_A2A = int(os.environ.get("DUMMY_A2A", "85"))

BF16_NP = ml_dtypes.bfloat16


def _ensure_ntff_hook():
    """The agent image lacks antenv.axon_hooks; recreate it so trace=True works."""
    if "antenv.axon_hooks" not in sys.modules:
        mod = types.ModuleType("antenv.axon_hooks")
        mod._hook = None
        def set_axon_ntff_profile_hook(h):
            mod._hook = h
        def get_axon_ntff_profile_hook():
            return mod._hook
        mod.set_axon_ntff_profile_hook = set_axon_ntff_profile_hook
        mod.get_axon_ntff_profile_hook = get_axon_ntff_profile_hook
        sys.modules["antenv.axon_hooks"] = mod
    m = sys.modules["antenv.axon_hooks"]
    if m.get_axon_ntff_profile_hook() is None:
        try:
            from trn_agent_boot.trn_boot import _ntff_profile_via_ctypes
            m.set_axon_ntff_profile_hook(
                _ntff_profile_via_ctypes("/opt/axon/libaxon_pjrt.so"))
        except Exception:
            pass


def build_graph():
    import concourse.bass as bass
    import concourse.bacc as bacc
    import concourse.tile as tile
    import concourse.mybir as mybir

    dt = mybir.dt
    F32, BF16 = dt.float32, dt.bfloat16
    AF = mybir.ActivationFunctionType
    ALU = mybir.AluOpType
    RG = [list(range(NC))]

    nc = bacc.Bacc(None, target_bir_lowering=False, debug=False, num_devices=NC)

    # ------------------------------------------------------------ I/O
    x_in = nc.dram_tensor("x_c", [TPC, DIM], F32, kind="ExternalInput")
    wt_in = nc.dram_tensor("wt_c", [GT * 128, DIM], BF16, kind="ExternalInput")
    bias_in = nc.dram_tensor("bias_c", [128, GT], F32, kind="ExternalInput")
    pwt_in = nc.dram_tensor("pwt", [DIM, DIM], BF16, kind="ExternalInput")
    pbf_in = nc.dram_tensor("pbf", [128, DIM], BF16, kind="ExternalInput")
    idn_in = nc.dram_tensor("idn", [128, 128], BF16, kind="ExternalInput")
    tri_in = nc.dram_tensor("tri", [128, 512], BF16, kind="ExternalInput")
    emat_in = nc.dram_tensor("emat", [33, 128], BF16, kind="ExternalInput")
    out_dram = nc.dram_tensor("out_c", [TPC, DIM], F32, kind="ExternalOutput")

    with tile.TileContext(nc) as tc:
        with (
            tc.tile_pool(name="persist", bufs=1) as pers,
            tc.tile_pool(name="dram", bufs=1, space="DRAM") as dram,
        ):
            # ---------------- DRAM bounce buffers ----------------
            qa1_in = dram.tile([NC * 256, TPC], BF16)         # q+k AllToAll
            qa1_out = dram.tile([NC * 256, TPC], BF16)
            qa2_in = dram.tile([NC * 128, TPC], BF16)         # v AllToAll
            qa2_out = dram.tile([NC * 128, TPC], BF16)
            ao_in = dram.tile([NC * 128, TPC], BF16)          # attn-out AllToAll
            ao_out = dram.tile([NC * 128, TPC], BF16)

            # idn first: transposes need it early; it is tiny
            idn_sb = pers.tile([128, 128], BF16)
            nc.sync.dma_start(idn_sb[:], idn_in[:])

            # ================= P1: LayerNorm (token slice, natural) ========
            xn_sb = pers.tile([128, 4 * DIM], BF16)   # 4 token tiles side by side
            wt_sb = pers.tile([128, GT * DIM], BF16)  # gt-major, k-minor qkv weights
            with tc.tile_pool(name="ln", bufs=4) as lnp:
                # x tiles first on the DMA queue, then the 24 weight-tile DMAs
                xts = []
                for t in range(4):
                    xt = lnp.tile([128, DIM], F32, tag="xt", name=f"xt{t}")
                    nc.sync.dma_start(xt[:], x_in[128 * t:128 * (t + 1), :])
                    xts.append(xt)
                qk_gts = [gt for gt in range(GT) if gt % 3 != 2]
                v_gts = [gt for gt in range(GT) if gt % 3 == 2]
                for gt in qk_gts + v_gts:
                    # host pre-arranged: row block gt is the contiguous
                    # [128 partitions x 1024] SBUF image of that weight tile
                    nc.sync.dma_start(
                        wt_sb[:, DIM * gt:DIM * (gt + 1)],
                        wt_in[128 * gt:128 * (gt + 1), :])
                for t in range(4):
                    xt = xts[t]
                    nmu = lnp.tile([128, 1], F32, tag="nmu")
                    musum = lnp.tile([128, 1], F32, tag="musum")
                    nc.vector.reduce_sum(musum[:], xt[:], axis=mybir.AxisListType.X)
                    nc.vector.tensor_scalar_mul(nmu[:], musum[:], -1.0 / DIM)
                    sq_dump = lnp.tile([128, DIM], BF16, tag="sqd")
                    sumsq = lnp.tile([128, 1], F32, tag="sumsq")
                    nc.scalar.activation(sq_dump[:], xt[:], AF.Square,
                                         bias=nmu[:], scale=1.0,
                                         accum_out=sumsq[:])
                    vareps = lnp.tile([128, 1], F32, tag="vareps")
                    nc.vector.tensor_scalar(vareps[:], sumsq[:], 1.0 / DIM, EPS,
                                            op0=ALU.mult, op1=ALU.add)
                    std = lnp.tile([128, 1], F32, tag="std")
                    nc.scalar.activation(std[:], vareps[:], AF.Sqrt)
                    rstd = lnp.tile([128, 1], F32, tag="rstd")
                    nc.vector.reciprocal(rstd[:], std[:])
                    nmr = lnp.tile([128, 1], F32, tag="nmr")
                    nc.vector.scalar_tensor_tensor(
                        nmr[:], nmu[:], 1.0, rstd[:],
                        op0=ALU.mult, op1=ALU.mult)
                    nc.scalar.activation(xn_sb[:, DIM * t:DIM * (t + 1)], xt[:],
                                         AF.Identity, bias=nmr[:], scale=rstd[:])

            # ================= P2: transpose xn -> xnT =====================
            xnT_sb = pers.tile([128, KT8 * TPC], BF16)  # [dim-tile partition, k*512+t128]
            with tc.tile_pool(name="ps_tr", bufs=6, space="PSUM") as pstr:
                for t in range(4):
                    for k in range(KT8):
                        trp = pstr.tile([128, 128], BF16, tag="tr")
                        nc.tensor.transpose(
                            trp[:], xn_sb[:, DIM * t + 128 * k: DIM * t + 128 * (k + 1)],
                            idn_sb[:])
                        nc.vector.tensor_copy(
                            xnT_sb[:, TPC * k + 128 * t: TPC * k + 128 * (t + 1)],
                            trp[:])

            # ---------------- other weight loads (background) -------------
            bias_sb = pers.tile([128, GT], F32)
            nc.sync.dma_start(bias_sb[:], bias_in[:])
            pwt_sb = pers.tile([128, KT8 * DIM], BF16)      # k-major proj weights
            nc.sync.dma_start(
                pwt_sb[:].rearrange("p (k o) -> p k o", o=DIM),
                pwt_in[:].rearrange("(k p) o -> p k o", p=128),
            )
            pbf_sb = pers.tile([128, DIM], BF16)
            nc.sync.dma_start(pbf_sb[:], pbf_in[:])
            tri_sb = pers.tile([128, 512], BF16)
            nc.sync.dma_start(tri_sb[:], tri_in[:])
            emat_sb = pers.tile([33, 128], BF16)
            nc.sync.dma_start(emat_sb[:], emat_in[:])
            sums_col = pers.tile([33, 512], F32)
            nc.vector.memset(sums_col[:], 1.0)

            # ================= P3: local token-parallel QKV ================
            # All 3072 qkv rows for this core's 512 tokens; rows are ordered
            # destination-core-major on the host, so row block 128*gt is the
            # (gt%3)-th third of chunk r=gt//3 of the AllToAll input.
            qkvL = pers.tile([128, GT * TPC], BF16)
            with tc.tile_pool(name="ps_q", bufs=3, space="PSUM") as psq:
                def emit_qkv(gt):
                    psg = psq.tile([128, TPC], F32, tag="q")
                    for k in range(KT8):
                        nc.tensor.matmul(
                            psg[:],
                            wt_sb[:, DIM * gt + 128 * k: DIM * gt + 128 * (k + 1)],
                            xnT_sb[:, TPC * k:TPC * (k + 1)],
                            start=(k == 0), stop=(k == KT8 - 1))
                    nc.vector.tensor_scalar(
                        qkvL[:, TPC * gt:TPC * (gt + 1)], psg[:],
                        bias_sb[:, gt:gt + 1], None, op0=ALU.add)
                    r, c = gt // 3, gt % 3
                    if c == 2:
                        dst = qa2_in[128 * r:128 * (r + 1), :]
                    else:
                        dst = qa1_in[256 * r + 128 * c: 256 * r + 128 * (c + 1), :]
                    nc.sync.dma_start(dst, qkvL[:, TPC * gt:TPC * (gt + 1)])

                # q+k tiles first -> AllToAll #1 overlaps the v tiles' matmuls
                for gt in qk_gts:
                    emit_qkv(gt)
                nc.gpsimd.collective_compute(
                    "AllToAll", ALU.bypass, replica_groups=RG,
                    ins=[qa1_in[:].opt()], outs=[qa1_out[:].opt()],
                )
                for gt in v_gts:
                    emit_qkv(gt)
                nc.gpsimd.collective_compute(
                    "AllToAll", ALU.bypass, replica_groups=RG,
                    ins=[qa2_in[:].opt()], outs=[qa2_out[:].opt()],
                )

            # -------- HAM keep-warm: dummy matmuls while the A2As are in flight
            with tc.tile_pool(name="ps_dummy", bufs=1, space="PSUM") as psd:
                dps = psd.tile([128, 512], F32, tag="d")
                for i in range(DUMMY_QA2A):
                    nc.tensor.matmul(dps[:], idn_sb[:],
                                     xnT_sb[:, 512 * (i % 8):512 * (i % 8) + 512],
                                     start=True, stop=True)

            # ================= P5/P6: head-parallel attention ==============
            qkvT = []
            for name in ("qT", "kT", "vT"):
                t_ = pers.tile([128, TOK], BF16, name=name)
                qkvT.append(t_)
            qT_sb, kT_sb, vT_sb = qkvT
            vnat = []
            for b in range(B):
                vb = pers.tile([128, 16 * 130], BF16, name=f"vnat{b}")
                nc.vector.memset(
                    vb[:].rearrange("p (j a w) -> p j a w", a=2, w=65)[:, :, :, 64:65], 1.0)
                vnat.append(vb)
            attnT = pers.tile([128, TOK], BF16)

            with (
                tc.tile_pool(name="pt", bufs=3) as ptp,
                tc.tile_pool(name="ps_s", bufs=3, space="PSUM") as pss,
                tc.tile_pool(name="ps_pv", bufs=4, space="PSUM") as psp,
                tc.tile_pool(name="ps_bc", bufs=1, space="PSUM") as psb,
                tc.tile_pool(name="sm", bufs=2) as smp,
            ):
                def emit_qload(s):
                    # my-heads q/k/v for source-core s's 512 tokens
                    nc.sync.dma_start(qT_sb[:, TPC * s:TPC * (s + 1)],
                                      qa1_out[256 * s: 256 * s + 128, :])
                    nc.sync.dma_start(kT_sb[:, TPC * s:TPC * (s + 1)],
                                      qa1_out[256 * s + 128: 256 * s + 256, :])
                    nc.sync.dma_start(vT_sb[:, TPC * s:TPC * (s + 1)],
                                      qa2_out[128 * s: 128 * s + 128, :])

                def emit_vtr(b, j):
                    vtr = psb.tile([128, 128], BF16, tag="bc")
                    nc.tensor.transpose(
                        vtr[:],
                        vT_sb[:, b * T + 128 * j: b * T + 128 * (j + 1)],
                        idn_sb[:])
                    nc.vector.tensor_copy(
                        vnat[b][:, 130 * j: 130 * j + 64], vtr[:, 0:64])
                    nc.vector.tensor_copy(
                        vnat[b][:, 130 * j + 65: 130 * j + 129], vtr[:, 64:128])

                def emit_attention(b, inject):
                    ii = 0
                    for qc in range(4):
                        q0 = b * T + 512 * qc
                        r = 4 * b + qc
                        pvA = psp.tile([65, 512], F32, tag="pv")
                        pvB = psp.tile([65, 512], F32, tag="pv")
                        nkp = 4 * qc + 4
                        pend = None
                        for kp in range(nkp):
                            k0 = b * T + 128 * kp
                            j = kp - 4 * qc       # >=0 -> diagonal block
                            o = 128 * j if j > 0 else 0
                            n = 512 - o
                            sA = pss.tile([128, 512], F32, tag="s")
                            sB = pss.tile([128, 512], F32, tag="s")
                            nc.tensor.matmul(sA[:, o:512], kT_sb[0:64, k0:k0 + 128],
                                             qT_sb[0:64, q0 + o:q0 + 512],
                                             start=True, stop=True)
                            nc.tensor.matmul(sB[:, o:512], kT_sb[64:128, k0:k0 + 128],
                                             qT_sb[64:128, q0 + o:q0 + 512],
                                             start=True, stop=True)
                            if pend is not None:
                                pkp, ppA, ppB, po = pend
                                nc.tensor.matmul(pvA[:, po:512],
                                                 vnat[b][:, 130 * pkp:130 * pkp + 65],
                                                 ppA[:, po:512],
                                                 start=(pkp == 0), stop=False,
                                                 skip_group_check=True)
                                nc.tensor.matmul(pvB[:, po:512],
                                                 vnat[b][:, 130 * pkp + 65:130 * pkp + 130],
                                                 ppB[:, po:512],
                                                 start=(pkp == 0), stop=False,
                                                 skip_group_check=True)
                            pA = ptp.tile([128, 512], BF16, tag="pA")
                            pB = ptp.tile([128, 512], BF16, tag="pB")
                            nc.scalar.activation(pA[:, o:512], sA[:, o:512], AF.Exp)
                            nc.scalar.activation(pB[:, o:512], sB[:, o:512], AF.Exp)
                            if j >= 0:
                                nc.vector.tensor_tensor(
                                    pA[:, o:512], pA[:, o:512], tri_sb[:, 0:n],
                                    op=ALU.mult)
                                nc.gpsimd.tensor_tensor(
                                    pB[:, o:512], pB[:, o:512], tri_sb[:, 0:n],
                                    op=ALU.mult)
                            pend = (kp, pA, pB, o)
                            if ii < len(inject):
                                inject[ii]()
                                ii += 1
                        pkp, ppA, ppB, po = pend
                        nc.tensor.matmul(pvA[:, po:512],
                                         vnat[b][:, 130 * pkp:130 * pkp + 65],
                                         ppA[:, po:512],
                                         start=(pkp == 0), stop=True,
                                         skip_group_check=True)
                        nc.tensor.matmul(pvB[:, po:512],
                                         vnat[b][:, 130 * pkp + 65:130 * pkp + 130],
                                         ppB[:, po:512],
                                         start=(pkp == 0), stop=True,
                                         skip_group_check=True)
                        nc.vector.tensor_copy(sums_col[0:1, :], pvA[64:65, :])
                        nc.vector.tensor_copy(sums_col[32:33, :], pvB[64:65, :])
                        rec = smp.tile([33, 512], F32, tag="rec")
                        nc.vector.reciprocal_approx_fast(rec[:], sums_col[:])
                        recb = smp.tile([33, 512], BF16, tag="recb")
                        nc.vector.tensor_copy(recb[:], rec[:])
                        bc2 = psb.tile([128, 512], F32, tag="bc")
                        nc.tensor.matmul(bc2[:], emat_sb[:], recb[:],
                                         start=True, stop=True)
                        bc2s = smp.tile([128, 512], BF16, tag="bc2s")
                        nc.vector.tensor_copy(bc2s[:], bc2[:])
                        nc.vector.tensor_tensor(
                            attnT[0:64, q0:q0 + 512], pvA[0:64, :],
                            bc2s[0:64, :], op=ALU.mult)
                        nc.vector.tensor_tensor(
                            attnT[64:128, q0:q0 + 512], pvB[0:64, :],
                            bc2s[64:128, :], op=ALU.mult)
                        nc.sync.dma_start(ao_in[128 * r:128 * (r + 1), :],
                                          attnT[:, TPC * r:TPC * (r + 1)])
                    while ii < len(inject):
                        inject[ii]()
                        ii += 1

                emit_qload(0)
                for j in range(4):
                    emit_vtr(0, j)
                inject_b0 = [
                    lambda: emit_qload(1),
                    lambda: emit_vtr(0, 4),
                    lambda: emit_vtr(0, 5),
                    lambda: emit_vtr(0, 6),
                    lambda: emit_vtr(0, 7),
                    lambda: emit_qload(2),
                    lambda: emit_vtr(0, 8),
                    lambda: emit_vtr(0, 9),
                    lambda: emit_vtr(0, 10),
                    lambda: emit_vtr(0, 11),
                    lambda: emit_qload(3),
                    lambda: emit_vtr(0, 12),
                    lambda: emit_vtr(0, 13),
                    lambda: emit_vtr(0, 14),
                    lambda: emit_vtr(0, 15),
                    lambda: emit_qload(4),
                    lambda: emit_qload(5),
                    lambda: emit_qload(6),
                    lambda: emit_qload(7),
                ] + [
                    (lambda jj=j: emit_vtr(1, jj)) for j in range(16)
                ]
                emit_attention(0, inject_b0)
                emit_attention(1, [])

            # ================= P7: AllToAll attention outputs ==============
            # (per-qc staging DMAs already issued inside emit_attention)
            nc.gpsimd.collective_compute(
                "AllToAll", ALU.bypass, replica_groups=RG,
                ins=[ao_in[:].opt()], outs=[ao_out[:].opt()],
            )

            # -------- HAM keep-warm: dummy matmuls while the A2A is in flight
            with tc.tile_pool(name="ps_dummy2", bufs=1, space="PSUM") as psd2:
                dps2 = psd2.tile([128, 512], F32, tag="d2")
                for i in range(DUMMY_A2A):
                    nc.tensor.matmul(dps2[:], idn_sb[:],
                                     attnT[:, 512 * (i % 8):512 * (i % 8) + 512],
                                     start=True, stop=True)

            # ================= P8: output projection (token slice) =========
            with (
                tc.tile_pool(name="projx", bufs=1) as pxp,
                tc.tile_pool(name="ps_o", bufs=4, space="PSUM") as pso,
                tc.tile_pool(name="outp", bufs=2) as outp,
            ):
                aT = []
                for ck in range(KT8):
                    ak = pxp.tile([128, TPC], BF16, tag=f"aT{ck}", name=f"ak{ck}")
                    nc.sync.dma_start(ak[:],
                                      ao_out[128 * ck:128 * (ck + 1), :])
                    aT.append(ak)
                for tt in range(4):
                    ps0 = pso.tile([128, 512], F32, tag="po")
                    ps1 = pso.tile([128, 512], F32, tag="po")
                    for ck in range(KT8):
                        lh = aT[ck][:, 128 * tt: 128 * (tt + 1)]
                        nc.tensor.matmul(ps0[:], lh,
                                         pwt_sb[:, DIM * ck: DIM * ck + 512],
                                         start=(ck == 0), stop=(ck == KT8 - 1))
                        nc.tensor.matmul(ps1[:], lh,
                                         pwt_sb[:, DIM * ck + 512: DIM * ck + 1024],
                                         start=(ck == 0), stop=(ck == KT8 - 1))
                    ot = outp.tile([128, DIM], F32, tag="ot")
                    nc.vector.tensor_tensor(ot[:, 0:512], ps0[:],
                                            pbf_sb[:, 0:512], op=ALU.add)
                    nc.vector.tensor_tensor(ot[:, 512:1024], ps1[:],
                                            pbf_sb[:, 512:1024], op=ALU.add)
                    nc.sync.dma_start(out_dram[128 * tt:128 * (tt + 1), :], ot[:])

    nc.compile()
    return nc


def host_prep(inputs):
    x = np.asarray(inputs["x"], np.float32).reshape(TOK, DIM)
    ln_w = np.asarray(inputs["ln_w"], np.float32)
    ln_b = np.asarray(inputs["ln_b"], np.float32)
    qkv_w = np.asarray(inputs["qkv_w"], np.float32)
    qkv_b = np.asarray(inputs["qkv_b"], np.float32)
    proj_w = np.asarray(inputs["proj_w"], np.float32)
    proj_b = np.asarray(inputs["proj_b"], np.float32)

    # fold LN affine into qkv weights; fold 1/sqrt(D) into Q rows
    Wp = qkv_w * ln_w[None, :]
    bp = qkv_b + qkv_w @ ln_b
    Wp[0:DIM] *= D ** -0.5
    bp[0:DIM] *= D ** -0.5

    # destination-core-major row permutation: for core c, its 384 rows are
    # [q(h2c), q(h2c+1), k(h2c), k(h2c+1), v(h2c), v(h2c+1)]
    rows = []
    for c in range(NC):
        for blk in range(3):
            for h in (2 * c, 2 * c + 1):
                rows.extend(range(blk * DIM + h * D, blk * DIM + (h + 1) * D))
    rows = np.array(rows)
    Wperm = Wp[rows]                      # [3072, 1024]
    bperm = bp[rows]                      # [3072]

    idn = np.eye(128, dtype=np.float32).astype(BF16_NP)
    tri = (np.arange(512)[None, :] >= np.arange(128)[:, None]).astype(BF16_NP)
    emat = np.zeros((33, 128), np.float32)
    emat[0, 0:64] = 1.0
    emat[32, 64:128] = 1.0
    emat = emat.astype(BF16_NP)
    pwt = proj_w.T.copy().astype(BF16_NP)
    pbf = np.broadcast_to(proj_b.reshape(1, DIM), (128, DIM)).copy().astype(BF16_NP)
    # SBUF image per gt-tile: wt_c[128*gt+p, 128*k+o] = Wperm[128*gt+o, 128*k+p]
    wt_c = np.ascontiguousarray(
        Wperm.reshape(GT, 128, KT8, 128).transpose(0, 3, 2, 1)
        .reshape(GT * 128, DIM)).astype(BF16_NP)
    bias_c = np.ascontiguousarray(bperm.reshape(GT, 128).T)    # [128, 24]

    in_maps = []
    for c in range(NC):
        in_maps.append(dict(
            x_c=np.ascontiguousarray(x[TPC * c:TPC * (c + 1)]),
            wt_c=wt_c, bias_c=bias_c,
            pwt=pwt, pbf=pbf, idn=idn, tri=tri, emat=emat,
        ))
    return in_maps


_CACHED = {}


def kernel(**inputs) -> np.ndarray:
    _ensure_ntff_hook()
    from concourse import bass_utils
    if TRACE:
        bass_utils.upload_artifacts = lambda tmpdir: "/tmp/noupload"

    if "nc" not in _CACHED:
        _CACHED["nc"] = build_graph()
    nc = _CACHED["nc"]

    in_maps = host_prep(inputs)
    res = bass_utils.run_bass_kernel_spmd(
        nc, in_maps, core_ids=list(range(NC)), trace=TRACE,
        trace_cores=list(range(NC)) if TRACE else None)
    _CACHED["last_result"] = res
    out = np.concatenate([res.results[c]["out_c"] for c in range(NC)], axis=0)
    return out.reshape(B, T, DIM).astype(np.float32)


# revision 44
# speedup vs baseline: 1.1185x; 1.0253x over previous
"""Distributed Trainium2 Bass kernel for fused LayerNorm + causal multi-head
attention + output projection (B=2, T=2048, DIM=1024, H=16, D=64) on 8 cores.

Sharding (v6):
  - LayerNorm + QKV projection + final projection: token-parallel
    (512 tokens/core). QKV is computed on LOCAL data (full 3072-row weight)
    BEFORE any collective, so the first-collective rendezvous (launch skew)
    is absorbed by ~60us of real matmul work instead of idle waiting.
  - qkv travels via one bf16 AllToAll into head-parallel layout
    (2 heads x 2 batches per core); attention outputs return via a second
    bf16 AllToAll; projection is token-parallel again.
  - causal diagonal blocks are N-trimmed; triangular mask via precomputed
    bf16 multiply (DVE+Pool); denominators via the vnat ones-column trick.

Compute dtype: bf16 matmuls with fp32 PSUM accumulation (rel err ~5e-3).
LN affine params and the 1/sqrt(D) score scale are folded into the QKV
weights on the host.
"""
import os
import sys
import types
import numpy as np
import ml_dtypes

# ---------------------------------------------------------------- constants
B, T, DIM, D = 2, 2048, 1024, 64
H = DIM // D            # 16 heads
NC = 8                  # cores
TOK = B * T             # 4096 tokens
TPC = TOK // NC         # 512 tokens per core
KT8 = DIM // 128        # 8 contraction tiles
GT = 3 * DIM // 128     # 24 qkv output tiles of 128 rows
EPS = 1e-5

TRACE = bool(int(os.environ.get("BASS_KERNEL_TRACE", "0")))
DUMMY_QA2A = int(os.environ.get("DUMMY_QA2A", "65"))
DUMMY_A2A = int(os.environ.get("DUMMY_A2A", "85"))

BF16_NP = ml_dtypes.bfloat16


def _ensure_ntff_hook():
    """The agent image lacks antenv.axon_hooks; recreate it so trace=True works."""
    if "antenv.axon_hooks" not in sys.modules:
        mod = types.ModuleType("antenv.axon_hooks")
        mod._hook = None
        def set_axon_ntff_profile_hook(h):
            mod._hook = h
        def get_axon_ntff_profile_hook():
            return mod._hook
        mod.set_axon_ntff_profile_hook = set_axon_ntff_profile_hook
        mod.get_axon_ntff_profile_hook = get_axon_ntff_profile_hook
        sys.modules["antenv.axon_hooks"] = mod
    m = sys.modules["antenv.axon_hooks"]
    if m.get_axon_ntff_profile_hook() is None:
        try:
            from trn_agent_boot.trn_boot import _ntff_profile_via_ctypes
            m.set_axon_ntff_profile_hook(
                _ntff_profile_via_ctypes("/opt/axon/libaxon_pjrt.so"))
        except Exception:
            pass


def build_graph():
    import concourse.bass as bass
    import concourse.bacc as bacc
    import concourse.tile as tile
    import concourse.mybir as mybir

    dt = mybir.dt
    F32, BF16 = dt.float32, dt.bfloat16
    AF = mybir.ActivationFunctionType
    ALU = mybir.AluOpType
    RG = [list(range(NC))]

    nc = bacc.Bacc(None, target_bir_lowering=False, debug=False, num_devices=NC)

    # ------------------------------------------------------------ I/O
    x_in = nc.dram_tensor("x_c", [TPC, DIM], F32, kind="ExternalInput")
    wt_in = nc.dram_tensor("wt_c", [GT * 128, DIM], BF16, kind="ExternalInput")
    bias_in = nc.dram_tensor("bias_c", [128, GT], F32, kind="ExternalInput")
    pwt_in = nc.dram_tensor("pwt", [DIM, DIM], BF16, kind="ExternalInput")
    pbf_in = nc.dram_tensor("pbf", [128, DIM], BF16, kind="ExternalInput")
    idn_in = nc.dram_tensor("idn", [128, 128], BF16, kind="ExternalInput")
    tri_in = nc.dram_tensor("tri", [128, 512], BF16, kind="ExternalInput")
    emat_in = nc.dram_tensor("emat", [33, 128], BF16, kind="ExternalInput")
    out_dram = nc.dram_tensor("out_c", [TPC, DIM], F32, kind="ExternalOutput")

    with tile.TileContext(nc) as tc:
        with (
            tc.tile_pool(name="persist", bufs=1) as pers,
            tc.tile_pool(name="dram", bufs=1, space="DRAM") as dram,
        ):
            # ---------------- DRAM bounce buffers ----------------
            qa1_in = dram.tile([NC * 256, TPC], BF16)         # q+k AllToAll
            qa1_out = dram.tile([NC * 256, TPC], BF16)
            qa2_in = dram.tile([NC * 128, TPC], BF16)         # v AllToAll
            qa2_out = dram.tile([NC * 128, TPC], BF16)
            ao_in = dram.tile([NC * 128, TPC], BF16)          # attn-out AllToAll
            ao_out = dram.tile([NC * 128, TPC], BF16)

            # idn first: transposes need it early; it is tiny
            idn_sb = pers.tile([128, 128], BF16)
            nc.sync.dma_start(idn_sb[:], idn_in[:])

            # ================= P1: LayerNorm (token slice, natural) ========
            xn_sb = pers.tile([128, 4 * DIM], BF16)   # 4 token tiles side by side
            wt_sb = pers.tile([128, GT * DIM], BF16)  # gt-major, k-minor qkv weights
            with tc.tile_pool(name="ln", bufs=4) as lnp:
                # x tiles first on the DMA queue, then the 24 weight-tile DMAs
                xts = []
                for t in range(4):
                    xt = lnp.tile([128, DIM], F32, tag="xt", name=f"xt{t}")
                    nc.sync.dma_start(xt[:], x_in[128 * t:128 * (t + 1), :])
                    xts.append(xt)
                qk_gts = [gt for gt in range(GT) if gt % 3 != 2]
                v_gts = [gt for gt in range(GT) if gt % 3 == 2]
                for gt in qk_gts + v_gts:
                    # host pre-arranged: row block gt is the contiguous
                    # [128 partitions x 1024] SBUF image of that weight tile
                    nc.sync.dma_start(
                        wt_sb[:, DIM * gt:DIM * (gt + 1)],
                        wt_in[128 * gt:128 * (gt + 1), :])
                for t in range(4):
                    xt = xts[t]
                    nmu = lnp.tile([128, 1], F32, tag="nmu")
                    musum = lnp.tile([128, 1], F32, tag="musum")
                    nc.vector.reduce_sum(musum[:], xt[:], axis=mybir.AxisListType.X)
                    nc.vector.tensor_scalar_mul(nmu[:], musum[:], -1.0 / DIM)
                    sq_dump = lnp.tile([128, DIM], BF16, tag="sqd")
                    sumsq = lnp.tile([128, 1], F32, tag="sumsq")
                    nc.scalar.activation(sq_dump[:], xt[:], AF.Square,
                                         bias=nmu[:], scale=1.0,
                                         accum_out=sumsq[:])
                    vareps = lnp.tile([128, 1], F32, tag="vareps")
                    nc.vector.tensor_scalar(vareps[:], sumsq[:], 1.0 / DIM, EPS,
                                            op0=ALU.mult, op1=ALU.add)
                    std = lnp.tile([128, 1], F32, tag="std")
                    nc.scalar.activation(std[:], vareps[:], AF.Sqrt)
                    rstd = lnp.tile([128, 1], F32, tag="rstd")
                    nc.vector.reciprocal(rstd[:], std[:])
                    nmr = lnp.tile([128, 1], F32, tag="nmr")
                    nc.vector.scalar_tensor_tensor(
                        nmr[:], nmu[:], 1.0, rstd[:],
                        op0=ALU.mult, op1=ALU.mult)
                    nc.scalar.activation(xn_sb[:, DIM * t:DIM * (t + 1)], xt[:],
                                         AF.Identity, bias=nmr[:], scale=rstd[:])

            # ================= P2: transpose xn -> xnT =====================
            xnT_sb = pers.tile([128, KT8 * TPC], BF16)  # [dim-tile partition, k*512+t128]
            with tc.tile_pool(name="ps_tr", bufs=6, space="PSUM") as pstr:
                for t in range(4):
                    for k in range(KT8):
                        trp = pstr.tile([128, 128], BF16, tag="tr")
                        nc.tensor.transpose(
                            trp[:], xn_sb[:, DIM * t + 128 * k: DIM * t + 128 * (k + 1)],
                            idn_sb[:])
                        nc.vector.tensor_copy(
                            xnT_sb[:, TPC * k + 128 * t: TPC * k + 128 * (t + 1)],
                            trp[:])

            # ---------------- other weight loads (background) -------------
            bias_sb = pers.tile([128, GT], F32)
            nc.sync.dma_start(bias_sb[:], bias_in[:])
            pwt_sb = pers.tile([128, KT8 * DIM], BF16)      # k-major proj weights
            nc.sync.dma_start(
                pwt_sb[:].rearrange("p (k o) -> p k o", o=DIM),
                pwt_in[:].rearrange("(k p) o -> p k o", p=128),
            )
            pbf_sb = pers.tile([128, DIM], BF16)
            nc.sync.dma_start(pbf_sb[:], pbf_in[:])
            tri_sb = pers.tile([128, 512], BF16)
            nc.sync.dma_start(tri_sb[:], tri_in[:])
            emat_sb = pers.tile([33, 128], BF16)
            nc.sync.dma_start(emat_sb[:], emat_in[:])
            sums_col = pers.tile([33, 512], F32)
            nc.vector.memset(sums_col[:], 1.0)

            # ================= P3: local token-parallel QKV ================
            # All 3072 qkv rows for this core's 512 tokens; rows are ordered
            # destination-core-major on the host, so row block 128*gt is the
            # (gt%3)-th third of chunk r=gt//3 of the AllToAll input.
            qkvL = pers.tile([128, GT * TPC], BF16)
            with tc.tile_pool(name="ps_q", bufs=3, space="PSUM") as psq:
                def emit_qkv(gt):
                    psg = psq.tile([128, TPC], F32, tag="q")
                    for k in range(KT8):
                        nc.tensor.matmul(
                            psg[:],
                            wt_sb[:, DIM * gt + 128 * k: DIM * gt + 128 * (k + 1)],
                            xnT_sb[:, TPC * k:TPC * (k + 1)],
                            start=(k == 0), stop=(k == KT8 - 1))
                    nc.vector.tensor_scalar(
                        qkvL[:, TPC * gt:TPC * (gt + 1)], psg[:],
                        bias_sb[:, gt:gt + 1], None, op0=ALU.add)
                    r, c = gt // 3, gt % 3
                    if c == 2:
                        dst = qa2_in[128 * r:128 * (r + 1), :]
                    else:
                        dst = qa1_in[256 * r + 128 * c: 256 * r + 128 * (c + 1), :]
                    nc.sync.dma_start(dst, qkvL[:, TPC * gt:TPC * (gt + 1)])

                # q+k tiles first -> AllToAll #1 overlaps the v tiles' matmuls
                for gt in qk_gts:
                    emit_qkv(gt)
                nc.gpsimd.collective_compute(
                    "AllToAll", ALU.bypass, replica_groups=RG,
                    ins=[qa1_in[:].opt()], outs=[qa1_out[:].opt()],
                )
                for gt in v_gts:
                    emit_qkv(gt)
                nc.gpsimd.collective_compute(
                    "AllToAll", ALU.bypass, replica_groups=RG,
                    ins=[qa2_in[:].opt()], outs=[qa2_out[:].opt()],
                )

            # -------- HAM keep-warm: dummy matmuls while the A2As are in flight
            with tc.tile_pool(name="ps_dummy", bufs=1, space="PSUM") as psd:
                dps = psd.tile([128, 512], F32, tag="d")
                for i in range(DUMMY_QA2A):
                    nc.tensor.matmul(dps[:], idn_sb[:],
                                     xnT_sb[:, 512 * (i % 8):512 * (i % 8) + 512],
                                     start=True, stop=True)

            # ================= P5/P6: head-parallel attention ==============
            qkvT = []
            for name in ("qT", "kT", "vT"):
                t_ = pers.tile([128, TOK], BF16, name=name)
                qkvT.append(t_)
            qT_sb, kT_sb, vT_sb = qkvT
            vnat = []
            for b in range(B):
                vb = pers.tile([128, 16 * 130], BF16, name=f"vnat{b}")
                nc.vector.memset(
                    vb[:].rearrange("p (j a w) -> p j a w", a=2, w=65)[:, :, :, 64:65], 1.0)
                vnat.append(vb)
            attnT = pers.tile([128, TOK], BF16)

            with (
                tc.tile_pool(name="pt", bufs=5) as ptp,
                tc.tile_pool(name="ps_s", bufs=3, space="PSUM") as pss,
                tc.tile_pool(name="ps_pv", bufs=4, space="PSUM") as psp,
                tc.tile_pool(name="ps_bc", bufs=1, space="PSUM") as psb,
                tc.tile_pool(name="sm", bufs=2) as smp,
            ):
                def emit_qload_qk(s):
                    # my-heads q/k for source-core s's 512 tokens (A2A#1)
                    nc.sync.dma_start(qT_sb[:, TPC * s:TPC * (s + 1)],
                                      qa1_out[256 * s: 256 * s + 128, :])
                    nc.sync.dma_start(kT_sb[:, TPC * s:TPC * (s + 1)],
                                      qa1_out[256 * s + 128: 256 * s + 256, :])

                def emit_qload_v(s):
                    # my-heads v (A2A#2) — issued only after all needed qk
                    # loads so its wait does not clog the DMA queue
                    nc.sync.dma_start(vT_sb[:, TPC * s:TPC * (s + 1)],
                                      qa2_out[128 * s: 128 * s + 128, :])

                def emit_vtr(b, j):
                    vtr = psb.tile([128, 128], BF16, tag="bc")
                    nc.tensor.transpose(
                        vtr[:],
                        vT_sb[:, b * T + 128 * j: b * T + 128 * (j + 1)],
                        idn_sb[:])
                    nc.vector.tensor_copy(
                        vnat[b][:, 130 * j: 130 * j + 64], vtr[:, 0:64])
                    nc.vector.tensor_copy(
                        vnat[b][:, 130 * j + 65: 130 * j + 129], vtr[:, 64:128])

                def emit_attention(b, inject, defer0=None):
                    ii = 0
                    for qc in range(4):
                        q0 = b * T + 512 * qc
                        r = 4 * b + qc
                        pvA = psp.tile([65, 512], F32, tag="pv")
                        pvB = psp.tile([65, 512], F32, tag="pv")
                        nkp = 4 * qc + 4
                        # defer PV for the first chunk: its QK/exp stream can
                        # then run in the shadow of the v AllToAll without any
                        # vnat-dependent PE op blocking the in-order queue.
                        defer = defer0 is not None and qc == 0
                        pend = None
                        pends = []

                        def flush(p4, stop):
                            pkp, ppA, ppB, po = p4
                            nc.tensor.matmul(pvA[:, po:512],
                                             vnat[b][:, 130 * pkp:130 * pkp + 65],
                                             ppA[:, po:512],
                                             start=(pkp == 0), stop=stop,
                                             skip_group_check=True)
                            nc.tensor.matmul(pvB[:, po:512],
                                             vnat[b][:, 130 * pkp + 65:130 * pkp + 130],
                                             ppB[:, po:512],
                                             start=(pkp == 0), stop=stop,
                                             skip_group_check=True)

                        for kp in range(nkp):
                            k0 = b * T + 128 * kp
                            j = kp - 4 * qc       # >=0 -> diagonal block
                            o = 128 * j if j > 0 else 0
                            n = 512 - o
                            sA = pss.tile([128, 512], F32, tag="s")
                            sB = pss.tile([128, 512], F32, tag="s")
                            nc.tensor.matmul(sA[:, o:512], kT_sb[0:64, k0:k0 + 128],
                                             qT_sb[0:64, q0 + o:q0 + 512],
                                             start=True, stop=True)
                            nc.tensor.matmul(sB[:, o:512], kT_sb[64:128, k0:k0 + 128],
                                             qT_sb[64:128, q0 + o:q0 + 512],
                                             start=True, stop=True)
                            if not defer and pend is not None:
                                flush(pend, stop=False)
                            pA = ptp.tile([128, 512], BF16, tag="pA")
                            pB = ptp.tile([128, 512], BF16, tag="pB")
                            nc.scalar.activation(pA[:, o:512], sA[:, o:512], AF.Exp)
                            nc.scalar.activation(pB[:, o:512], sB[:, o:512], AF.Exp)
                            if j >= 0:
                                nc.vector.tensor_tensor(
                                    pA[:, o:512], pA[:, o:512], tri_sb[:, 0:n],
                                    op=ALU.mult)
                                nc.gpsimd.tensor_tensor(
                                    pB[:, o:512], pB[:, o:512], tri_sb[:, 0:n],
                                    op=ALU.mult)
                            if defer:
                                pends.append((kp, pA, pB, o))
                            else:
                                pend = (kp, pA, pB, o)
                            if ii < len(inject):
                                inject[ii]()
                                ii += 1
                        if defer:
                            defer0()          # V transposes, now that QK is queued
                            for idx, p4 in enumerate(pends):
                                flush(p4, stop=(idx == len(pends) - 1))
                        else:
                            flush(pend, stop=True)
                        nc.vector.tensor_copy(sums_col[0:1, :], pvA[64:65, :])
                        nc.vector.tensor_copy(sums_col[32:33, :], pvB[64:65, :])
                        rec = smp.tile([33, 512], F32, tag="rec")
                        nc.vector.reciprocal_approx_fast(rec[:], sums_col[:])
                        recb = smp.tile([33, 512], BF16, tag="recb")
                        nc.vector.tensor_copy(recb[:], rec[:])
                        bc2 = psb.tile([128, 512], F32, tag="bc")
                        nc.tensor.matmul(bc2[:], emat_sb[:], recb[:],
                                         start=True, stop=True)
                        bc2s = smp.tile([128, 512], BF16, tag="bc2s")
                        nc.vector.tensor_copy(bc2s[:], bc2[:])
                        nc.vector.tensor_tensor(
                            attnT[0:64, q0:q0 + 512], pvA[0:64, :],
                            bc2s[0:64, :], op=ALU.mult)
                        nc.vector.tensor_tensor(
                            attnT[64:128, q0:q0 + 512], pvB[0:64, :],
                            bc2s[64:128, :], op=ALU.mult)
                        nc.sync.dma_start(ao_in[128 * r:128 * (r + 1), :],
                                          attnT[:, TPC * r:TPC * (r + 1)])
                    while ii < len(inject):
                        inject[ii]()
                        ii += 1

                emit_qload_qk(0)
                emit_qload_qk(1)
                inject_b0 = [
                    # qc0 slots (4): qk prefetch + v loads (v waits A2A#2
                    # on the DMA queue only, after all early qk loads)
                    lambda: emit_qload_qk(2),
                    lambda: emit_qload_qk(3),
                    lambda: emit_qload_v(0),
                    lambda: emit_qload_v(1),
                    # qc1 slots (8)
                    lambda: (emit_vtr(0, 4), emit_vtr(0, 5)),
                    lambda: (emit_vtr(0, 6), emit_vtr(0, 7)),
                    lambda: emit_qload_v(2),
                    lambda: emit_qload_v(3),
                    lambda: emit_qload_qk(4),
                    lambda: emit_qload_qk(5),
                    lambda: emit_qload_qk(6),
                    lambda: emit_qload_qk(7),
                    # qc2 slots (12)
                    lambda: (emit_vtr(0, 8), emit_vtr(0, 9)),
                    lambda: (emit_vtr(0, 10), emit_vtr(0, 11)),
                    lambda: (emit_vtr(0, 12), emit_vtr(0, 13)),
                    lambda: (emit_vtr(0, 14), emit_vtr(0, 15)),
                    lambda: emit_qload_v(4),
                    lambda: emit_qload_v(5),
                    lambda: emit_qload_v(6),
                    lambda: emit_qload_v(7),
                ] + [
                    (lambda jj=j: (emit_vtr(1, 2 * jj), emit_vtr(1, 2 * jj + 1)))
                    for j in range(8)
                ]
                emit_attention(0, inject_b0,
                               defer0=lambda: [emit_vtr(0, j) for j in range(4)])
                emit_attention(1, [])

            # ================= P7: AllToAll attention outputs ==============
            # (per-qc staging DMAs already issued inside emit_attention)
            nc.gpsimd.collective_compute(
                "AllToAll", ALU.bypass, replica_groups=RG,
                ins=[ao_in[:].opt()], outs=[ao_out[:].opt()],
            )

            # -------- HAM keep-warm: dummy matmuls while the A2A is in flight
            with tc.tile_pool(name="ps_dummy2", bufs=1, space="PSUM") as psd2:
                dps2 = psd2.tile([128, 512], F32, tag="d2")
                for i in range(DUMMY_A2A):
                    nc.tensor.matmul(dps2[:], idn_sb[:],
                                     attnT[:, 512 * (i % 8):512 * (i % 8) + 512],
                                     start=True, stop=True)

            # ================= P8: output projection (token slice) =========
            with (
                tc.tile_pool(name="projx", bufs=1) as pxp,
                tc.tile_pool(name="ps_o", bufs=4, space="PSUM") as pso,
                tc.tile_pool(name="outp", bufs=2) as outp,
            ):
                aT = []
                for ck in range(KT8):
                    ak = pxp.tile([128, TPC], BF16, tag=f"aT{ck}", name=f"ak{ck}")
                    nc.sync.dma_start(ak[:],
                                      ao_out[128 * ck:128 * (ck + 1), :])
                    aT.append(ak)
                for tt in range(4):
                    ps0 = pso.tile([128, 512], F32, tag="po")
                    ps1 = pso.tile([128, 512], F32, tag="po")
                    for ck in range(KT8):
                        lh = aT[ck][:, 128 * tt: 128 * (tt + 1)]
                        nc.tensor.matmul(ps0[:], lh,
                                         pwt_sb[:, DIM * ck: DIM * ck + 512],
                                         start=(ck == 0), stop=(ck == KT8 - 1))
                        nc.tensor.matmul(ps1[:], lh,
                                         pwt_sb[:, DIM * ck + 512: DIM * ck + 1024],
                                         start=(ck == 0), stop=(ck == KT8 - 1))
                    ot = outp.tile([128, DIM], F32, tag="ot")
                    nc.vector.tensor_tensor(ot[:, 0:512], ps0[:],
                                            pbf_sb[:, 0:512], op=ALU.add)
                    nc.vector.tensor_tensor(ot[:, 512:1024], ps1[:],
                                            pbf_sb[:, 512:1024], op=ALU.add)
                    nc.sync.dma_start(out_dram[128 * tt:128 * (tt + 1), :], ot[:])

    nc.compile()
    return nc


def host_prep(inputs):
    x = np.asarray(inputs["x"], np.float32).reshape(TOK, DIM)
    ln_w = np.asarray(inputs["ln_w"], np.float32)
    ln_b = np.asarray(inputs["ln_b"], np.float32)
    qkv_w = np.asarray(inputs["qkv_w"], np.float32)
    qkv_b = np.asarray(inputs["qkv_b"], np.float32)
    proj_w = np.asarray(inputs["proj_w"], np.float32)
    proj_b = np.asarray(inputs["proj_b"], np.float32)

    # fold LN affine into qkv weights; fold 1/sqrt(D) into Q rows
    Wp = qkv_w * ln_w[None, :]
    bp = qkv_b + qkv_w @ ln_b
    Wp[0:DIM] *= D ** -0.5
    bp[0:DIM] *= D ** -0.5

    # destination-core-major row permutation: for core c, its 384 rows are
    # [q(h2c), q(h2c+1), k(h2c), k(h2c+1), v(h2c), v(h2c+1)]
    rows = []
    for c in range(NC):
        for blk in range(3):
            for h in (2 * c, 2 * c + 1):
                rows.extend(range(blk * DIM + h * D, blk * DIM + (h + 1) * D))
    rows = np.array(rows)
    Wperm = Wp[rows]                      # [3072, 1024]
    bperm = bp[rows]                      # [3072]

    idn = np.eye(128, dtype=np.float32).astype(BF16_NP)
    tri = (np.arange(512)[None, :] >= np.arange(128)[:, None]).astype(BF16_NP)
    emat = np.zeros((33, 128), np.float32)
    emat[0, 0:64] = 1.0
    emat[32, 64:128] = 1.0
    emat = emat.astype(BF16_NP)
    pwt = proj_w.T.copy().astype(BF16_NP)
    pbf = np.broadcast_to(proj_b.reshape(1, DIM), (128, DIM)).copy().astype(BF16_NP)
    # SBUF image per gt-tile: wt_c[128*gt+p, 128*k+o] = Wperm[128*gt+o, 128*k+p]
    wt_c = np.ascontiguousarray(
        Wperm.reshape(GT, 128, KT8, 128).transpose(0, 3, 2, 1)
        .reshape(GT * 128, DIM)).astype(BF16_NP)
    bias_c = np.ascontiguousarray(bperm.reshape(GT, 128).T)    # [128, 24]

    in_maps = []
    for c in range(NC):
        in_maps.append(dict(
            x_c=np.ascontiguousarray(x[TPC * c:TPC * (c + 1)]),
            wt_c=wt_c, bias_c=bias_c,
            pwt=pwt, pbf=pbf, idn=idn, tri=tri, emat=emat,
        ))
    return in_maps


_CACHED = {}


def kernel(**inputs) -> np.ndarray:
    _ensure_ntff_hook()
    from concourse import bass_utils
    if TRACE:
        bass_utils.upload_artifacts = lambda tmpdir: "/tmp/noupload"

    if "nc" not in _CACHED:
        _CACHED["nc"] = build_graph()
    nc = _CACHED["nc"]

    in_maps = host_prep(inputs)
    res = bass_utils.run_bass_kernel_spmd(
        nc, in_maps, core_ids=list(range(NC)), trace=TRACE,
        trace_cores=list(range(NC)) if TRACE else None)
    _CACHED["last_result"] = res
    out = np.concatenate([res.results[c]["out_c"] for c in range(NC)], axis=0)
    return out.reshape(B, T, DIM).astype(np.float32)


# revision 48
# speedup vs baseline: 1.1739x; 1.0495x over previous
"""Distributed Trainium2 Bass kernel for fused LayerNorm + causal multi-head
attention + output projection (B=2, T=2048, DIM=1024, H=16, D=64) on 8 cores.

Sharding (v6):
  - LayerNorm + QKV projection + final projection: token-parallel
    (512 tokens/core). QKV is computed on LOCAL data (full 3072-row weight)
    BEFORE any collective, so the first-collective rendezvous (launch skew)
    is absorbed by ~60us of real matmul work instead of idle waiting.
  - qkv travels via one bf16 AllToAll into head-parallel layout
    (2 heads x 2 batches per core); attention outputs return via a second
    bf16 AllToAll; projection is token-parallel again.
  - causal diagonal blocks are N-trimmed; triangular mask via precomputed
    bf16 multiply (DVE+Pool); denominators via the vnat ones-column trick.

Compute dtype: bf16 matmuls with fp32 PSUM accumulation (rel err ~5e-3).
LN affine params and the 1/sqrt(D) score scale are folded into the QKV
weights on the host.
"""
import os
import sys
import types
import numpy as np
import ml_dtypes

# ---------------------------------------------------------------- constants
B, T, DIM, D = 2, 2048, 1024, 64
H = DIM // D            # 16 heads
NC = 8                  # cores
TOK = B * T             # 4096 tokens
TPC = TOK // NC         # 512 tokens per core
KT8 = DIM // 128        # 8 contraction tiles
GT = 3 * DIM // 128     # 24 qkv output tiles of 128 rows
EPS = 1e-5

TRACE = bool(int(os.environ.get("BASS_KERNEL_TRACE", "0")))
DUMMY_QA2A = int(os.environ.get("DUMMY_QA2A", "65"))
DUMMY_A2A = int(os.environ.get("DUMMY_A2A", "85"))

BF16_NP = ml_dtypes.bfloat16


def _ensure_ntff_hook():
    """The agent image lacks antenv.axon_hooks; recreate it so trace=True works."""
    if "antenv.axon_hooks" not in sys.modules:
        mod = types.ModuleType("antenv.axon_hooks")
        mod._hook = None
        def set_axon_ntff_profile_hook(h):
            mod._hook = h
        def get_axon_ntff_profile_hook():
            return mod._hook
        mod.set_axon_ntff_profile_hook = set_axon_ntff_profile_hook
        mod.get_axon_ntff_profile_hook = get_axon_ntff_profile_hook
        sys.modules["antenv.axon_hooks"] = mod
    m = sys.modules["antenv.axon_hooks"]
    if m.get_axon_ntff_profile_hook() is None:
        try:
            from trn_agent_boot.trn_boot import _ntff_profile_via_ctypes
            m.set_axon_ntff_profile_hook(
                _ntff_profile_via_ctypes("/opt/axon/libaxon_pjrt.so"))
        except Exception:
            pass


def build_graph():
    import concourse.bass as bass
    import concourse.bacc as bacc
    import concourse.tile as tile
    import concourse.mybir as mybir

    dt = mybir.dt
    F32, BF16 = dt.float32, dt.bfloat16
    AF = mybir.ActivationFunctionType
    ALU = mybir.AluOpType
    RG = [list(range(NC))]

    nc = bacc.Bacc(None, target_bir_lowering=False, debug=False, num_devices=NC)

    # ------------------------------------------------------------ I/O
    x_in = nc.dram_tensor("x_c", [TPC, DIM], F32, kind="ExternalInput")
    wt_in = nc.dram_tensor("wt_c", [GT * 128, DIM], BF16, kind="ExternalInput")
    bias_in = nc.dram_tensor("bias_c", [128, GT], F32, kind="ExternalInput")
    pwt_in = nc.dram_tensor("pwt", [DIM, DIM], BF16, kind="ExternalInput")
    pbf_in = nc.dram_tensor("pbf", [128, DIM], BF16, kind="ExternalInput")
    idn_in = nc.dram_tensor("idn", [128, 128], BF16, kind="ExternalInput")
    tri_in = nc.dram_tensor("tri", [128, 512], BF16, kind="ExternalInput")
    emat_in = nc.dram_tensor("emat", [33, 128], BF16, kind="ExternalInput")
    out_dram = nc.dram_tensor("out_c", [TPC, DIM], F32, kind="ExternalOutput")

    with tile.TileContext(nc) as tc:
        with (
            tc.tile_pool(name="persist", bufs=1) as pers,
            tc.tile_pool(name="dram", bufs=1, space="DRAM") as dram,
        ):
            # ---------------- DRAM bounce buffers ----------------
            qa1_in = dram.tile([NC * 256, TPC], BF16)         # q+k AllToAll
            qa1_out = dram.tile([NC * 256, TPC], BF16)
            qa2_in = dram.tile([NC * 128, TPC], BF16)         # v AllToAll
            qa2_out = dram.tile([NC * 128, TPC], BF16)
            ao_in = dram.tile([NC * 128, TPC], BF16)          # attn-out AllToAll
            ao_out = dram.tile([NC * 128, TPC], BF16)

            # idn first: transposes need it early; it is tiny
            idn_sb = pers.tile([128, 128], BF16)
            nc.sync.dma_start(idn_sb[:], idn_in[:])

            # ================= P1: LayerNorm (token slice, natural) ========
            xn_sb = pers.tile([128, 4 * DIM], BF16)   # 4 token tiles side by side
            wt_sb = pers.tile([128, GT * DIM], BF16)  # gt-major, k-minor qkv weights
            with tc.tile_pool(name="ln", bufs=4) as lnp:
                # x tiles first on the DMA queue, then the 24 weight-tile DMAs
                xts = []
                for t in range(4):
                    xt = lnp.tile([128, DIM], F32, tag="xt", name=f"xt{t}")
                    nc.sync.dma_start(xt[:], x_in[128 * t:128 * (t + 1), :])
                    xts.append(xt)
                qk_gts = [gt for gt in range(GT) if gt % 3 != 2]
                v_gts = [gt for gt in range(GT) if gt % 3 == 2]
                for gt in qk_gts + v_gts:
                    # host pre-arranged: row block gt is the contiguous
                    # [128 partitions x 1024] SBUF image of that weight tile
                    nc.sync.dma_start(
                        wt_sb[:, DIM * gt:DIM * (gt + 1)],
                        wt_in[128 * gt:128 * (gt + 1), :])
                for t in range(4):
                    xt = xts[t]
                    nmu = lnp.tile([128, 1], F32, tag="nmu")
                    musum = lnp.tile([128, 1], F32, tag="musum")
                    nc.vector.reduce_sum(musum[:], xt[:], axis=mybir.AxisListType.X)
                    nc.vector.tensor_scalar_mul(nmu[:], musum[:], -1.0 / DIM)
                    sq_dump = lnp.tile([128, DIM], BF16, tag="sqd")
                    sumsq = lnp.tile([128, 1], F32, tag="sumsq")
                    nc.scalar.activation(sq_dump[:], xt[:], AF.Square,
                                         bias=nmu[:], scale=1.0,
                                         accum_out=sumsq[:])
                    vareps = lnp.tile([128, 1], F32, tag="vareps")
                    nc.vector.tensor_scalar(vareps[:], sumsq[:], 1.0 / DIM, EPS,
                                            op0=ALU.mult, op1=ALU.add)
                    std = lnp.tile([128, 1], F32, tag="std")
                    nc.scalar.activation(std[:], vareps[:], AF.Sqrt)
                    rstd = lnp.tile([128, 1], F32, tag="rstd")
                    nc.vector.reciprocal(rstd[:], std[:])
                    nmr = lnp.tile([128, 1], F32, tag="nmr")
                    nc.vector.scalar_tensor_tensor(
                        nmr[:], nmu[:], 1.0, rstd[:],
                        op0=ALU.mult, op1=ALU.mult)
                    nc.scalar.activation(xn_sb[:, DIM * t:DIM * (t + 1)], xt[:],
                                         AF.Identity, bias=nmr[:], scale=rstd[:])

            # ================= P2: transpose xn -> xnT =====================
            xnT_sb = pers.tile([128, KT8 * TPC], BF16)  # [dim-tile partition, k*512+t128]
            with tc.tile_pool(name="ps_tr", bufs=6, space="PSUM") as pstr:
                for t in range(4):
                    for k in range(KT8):
                        trp = pstr.tile([128, 128], BF16, tag="tr")
                        nc.tensor.transpose(
                            trp[:], xn_sb[:, DIM * t + 128 * k: DIM * t + 128 * (k + 1)],
                            idn_sb[:])
                        nc.vector.tensor_copy(
                            xnT_sb[:, TPC * k + 128 * t: TPC * k + 128 * (t + 1)],
                            trp[:])

            # ---------------- other weight loads (background) -------------
            bias_sb = pers.tile([128, GT], F32)
            nc.sync.dma_start(bias_sb[:], bias_in[:])
            pwt_sb = pers.tile([128, KT8 * DIM], BF16)      # k-major proj weights
            nc.sync.dma_start(
                pwt_sb[:].rearrange("p (k o) -> p k o", o=DIM),
                pwt_in[:].rearrange("(k p) o -> p k o", p=128),
            )
            pbf_sb = pers.tile([128, DIM], BF16)
            nc.sync.dma_start(pbf_sb[:], pbf_in[:])
            tri_sb = pers.tile([128, 512], BF16)
            nc.sync.dma_start(tri_sb[:], tri_in[:])
            emat_sb = pers.tile([33, 128], BF16)
            nc.sync.dma_start(emat_sb[:], emat_in[:])
            sums_col = pers.tile([33, 512], F32)
            nc.vector.memset(sums_col[:], 1.0)

            # ================= P3: local token-parallel QKV ================
            # All 3072 qkv rows for this core's 512 tokens; rows are ordered
            # destination-core-major on the host, so row block 128*gt is the
            # (gt%3)-th third of chunk r=gt//3 of the AllToAll input.
            qkvL = pers.tile([128, GT * TPC], BF16)
            with tc.tile_pool(name="ps_q", bufs=3, space="PSUM") as psq:
                def emit_qkv(gt):
                    psg = psq.tile([128, TPC], F32, tag="q")
                    for k in range(KT8):
                        nc.tensor.matmul(
                            psg[:],
                            wt_sb[:, DIM * gt + 128 * k: DIM * gt + 128 * (k + 1)],
                            xnT_sb[:, TPC * k:TPC * (k + 1)],
                            start=(k == 0), stop=(k == KT8 - 1))
                    nc.vector.tensor_scalar(
                        qkvL[:, TPC * gt:TPC * (gt + 1)], psg[:],
                        bias_sb[:, gt:gt + 1], None, op0=ALU.add)
                    r, c = gt // 3, gt % 3
                    if c == 2:
                        dst = qa2_in[128 * r:128 * (r + 1), :]
                    else:
                        dst = qa1_in[256 * r + 128 * c: 256 * r + 128 * (c + 1), :]
                    nc.sync.dma_start(dst, qkvL[:, TPC * gt:TPC * (gt + 1)])

                # q+k tiles first -> AllToAll #1 overlaps the v tiles' matmuls
                for gt in qk_gts:
                    emit_qkv(gt)
                nc.gpsimd.collective_compute(
                    "AllToAll", ALU.bypass, replica_groups=RG,
                    ins=[qa1_in[:].opt()], outs=[qa1_out[:].opt()],
                )
                for gt in v_gts:
                    emit_qkv(gt)
                nc.gpsimd.collective_compute(
                    "AllToAll", ALU.bypass, replica_groups=RG,
                    ins=[qa2_in[:].opt()], outs=[qa2_out[:].opt()],
                )

            # -------- HAM keep-warm: dummy matmuls while the A2As are in flight
            with tc.tile_pool(name="ps_dummy", bufs=1, space="PSUM") as psd:
                dps = psd.tile([128, 512], F32, tag="d")
                for i in range(DUMMY_QA2A):
                    nc.tensor.matmul(dps[:], idn_sb[:],
                                     xnT_sb[:, 512 * (i % 8):512 * (i % 8) + 512],
                                     start=True, stop=True)

            # ================= P5/P6: head-parallel attention ==============
            qkvT = []
            for name in ("qT", "kT", "vT"):
                t_ = pers.tile([128, TOK], BF16, name=name)
                qkvT.append(t_)
            qT_sb, kT_sb, vT_sb = qkvT
            vnat = []
            for b in range(B):
                vb = pers.tile([128, 16 * 130], BF16, name=f"vnat{b}")
                nc.vector.memset(
                    vb[:].rearrange("p (j a w) -> p j a w", a=2, w=65)[:, :, :, 64:65], 1.0)
                vnat.append(vb)
            attnT = pers.tile([128, TOK], BF16)

            with (
                tc.tile_pool(name="pt", bufs=14) as ptp,
                tc.tile_pool(name="ps_s", bufs=3, space="PSUM") as pss,
                tc.tile_pool(name="ps_pv", bufs=4, space="PSUM") as psp,
                tc.tile_pool(name="ps_bc", bufs=1, space="PSUM") as psb,
                tc.tile_pool(name="sm", bufs=2) as smp,
            ):
                def emit_qload_qk(s):
                    # my-heads q/k for source-core s's 512 tokens (A2A#1)
                    nc.sync.dma_start(qT_sb[:, TPC * s:TPC * (s + 1)],
                                      qa1_out[256 * s: 256 * s + 128, :])
                    nc.sync.dma_start(kT_sb[:, TPC * s:TPC * (s + 1)],
                                      qa1_out[256 * s + 128: 256 * s + 256, :])

                def emit_qload_v(s):
                    # my-heads v (A2A#2) — issued only after all needed qk
                    # loads so its wait does not clog the DMA queue
                    nc.sync.dma_start(vT_sb[:, TPC * s:TPC * (s + 1)],
                                      qa2_out[128 * s: 128 * s + 128, :])

                def emit_vtr(b, j):
                    vtr = psb.tile([128, 128], BF16, tag="bc")
                    nc.tensor.transpose(
                        vtr[:],
                        vT_sb[:, b * T + 128 * j: b * T + 128 * (j + 1)],
                        idn_sb[:])
                    nc.vector.tensor_copy(
                        vnat[b][:, 130 * j: 130 * j + 64], vtr[:, 0:64])
                    nc.vector.tensor_copy(
                        vnat[b][:, 130 * j + 65: 130 * j + 129], vtr[:, 64:128])

                def emit_attention(b, inject, ndefer=0, defer_mid=None):
                    ii = 0
                    deferred = []
                    for qc in range(4):
                        q0 = b * T + 512 * qc
                        r = 4 * b + qc
                        pvA = psp.tile([65, 512], F32, tag="pv")
                        pvB = psp.tile([65, 512], F32, tag="pv")
                        nkp = 4 * qc + 4
                        # defer PV for the first ndefer chunks: their QK/exp
                        # streams then run in the shadow of the v AllToAll with
                        # no vnat-dependent PE op blocking the in-order queue.
                        defer = qc < ndefer
                        pend = None
                        pends = []

                        def flush(p4, stop, pvA=pvA, pvB=pvB):
                            pkp, ppA, ppB, po = p4
                            nc.tensor.matmul(pvA[:, po:512],
                                             vnat[b][:, 130 * pkp:130 * pkp + 65],
                                             ppA[:, po:512],
                                             start=(pkp == 0), stop=stop,
                                             skip_group_check=True)
                            nc.tensor.matmul(pvB[:, po:512],
                                             vnat[b][:, 130 * pkp + 65:130 * pkp + 130],
                                             ppB[:, po:512],
                                             start=(pkp == 0), stop=stop,
                                             skip_group_check=True)

                        def norm(pvA=pvA, pvB=pvB, q0=q0, r=r):
                            nc.vector.tensor_copy(sums_col[0:1, :], pvA[64:65, :])
                            nc.vector.tensor_copy(sums_col[32:33, :], pvB[64:65, :])
                            rec = smp.tile([33, 512], F32, tag="rec")
                            nc.vector.reciprocal_approx_fast(rec[:], sums_col[:])
                            recb = smp.tile([33, 512], BF16, tag="recb")
                            nc.vector.tensor_copy(recb[:], rec[:])
                            bc2 = psb.tile([128, 512], F32, tag="bc")
                            nc.tensor.matmul(bc2[:], emat_sb[:], recb[:],
                                             start=True, stop=True)
                            bc2s = smp.tile([128, 512], BF16, tag="bc2s")
                            nc.vector.tensor_copy(bc2s[:], bc2[:])
                            nc.vector.tensor_tensor(
                                attnT[0:64, q0:q0 + 512], pvA[0:64, :],
                                bc2s[0:64, :], op=ALU.mult)
                            nc.vector.tensor_tensor(
                                attnT[64:128, q0:q0 + 512], pvB[0:64, :],
                                bc2s[64:128, :], op=ALU.mult)
                            nc.sync.dma_start(ao_in[128 * r:128 * (r + 1), :],
                                              attnT[:, TPC * r:TPC * (r + 1)])

                        for kp in range(nkp):
                            k0 = b * T + 128 * kp
                            j = kp - 4 * qc       # >=0 -> diagonal block
                            o = 128 * j if j > 0 else 0
                            n = 512 - o
                            sA = pss.tile([128, 512], F32, tag="s")
                            sB = pss.tile([128, 512], F32, tag="s")
                            nc.tensor.matmul(sA[:, o:512], kT_sb[0:64, k0:k0 + 128],
                                             qT_sb[0:64, q0 + o:q0 + 512],
                                             start=True, stop=True)
                            nc.tensor.matmul(sB[:, o:512], kT_sb[64:128, k0:k0 + 128],
                                             qT_sb[64:128, q0 + o:q0 + 512],
                                             start=True, stop=True)
                            if not defer and pend is not None:
                                flush(pend, stop=False)
                            pA = ptp.tile([128, 512], BF16, tag="pA")
                            pB = ptp.tile([128, 512], BF16, tag="pB")
                            nc.scalar.activation(pA[:, o:512], sA[:, o:512], AF.Exp)
                            nc.scalar.activation(pB[:, o:512], sB[:, o:512], AF.Exp)
                            if j >= 0:
                                nc.vector.tensor_tensor(
                                    pA[:, o:512], pA[:, o:512], tri_sb[:, 0:n],
                                    op=ALU.mult)
                                nc.gpsimd.tensor_tensor(
                                    pB[:, o:512], pB[:, o:512], tri_sb[:, 0:n],
                                    op=ALU.mult)
                            if defer:
                                pends.append((kp, pA, pB, o))
                            else:
                                pend = (kp, pA, pB, o)
                            if ii < len(inject):
                                inject[ii]()
                                ii += 1
                        if defer:
                            deferred.append((flush, pends, norm))
                            if qc == ndefer - 1:
                                defer_mid()   # V transposes, now that QK is queued
                                for dflush, dpends, dnorm in deferred:
                                    for idx, p4 in enumerate(dpends):
                                        dflush(p4, stop=(idx == len(dpends) - 1))
                                    dnorm()
                        else:
                            flush(pend, stop=True)
                            norm()
                    while ii < len(inject):
                        inject[ii]()
                        ii += 1

                emit_qload_qk(0)
                emit_qload_qk(1)
                inject_b0 = [
                    # qc0+qc1 slots (12, DMA-only so the deferred QK stream
                    # is never blocked by a vnat-dependent PE op; v loads
                    # queue after all early qk loads)
                    lambda: emit_qload_qk(2),
                    lambda: emit_qload_qk(3),
                    lambda: emit_qload_v(0),
                    lambda: emit_qload_v(1),
                    lambda: emit_qload_v(2),
                    lambda: emit_qload_v(3),
                    lambda: emit_qload_qk(4),
                    lambda: emit_qload_qk(5),
                    lambda: emit_qload_qk(6),
                    lambda: emit_qload_qk(7),
                    lambda: None,
                    lambda: None,
                    # qc2 slots (12)
                    lambda: (emit_vtr(0, 8), emit_vtr(0, 9)),
                    lambda: (emit_vtr(0, 10), emit_vtr(0, 11)),
                    lambda: (emit_vtr(0, 12), emit_vtr(0, 13)),
                    lambda: (emit_vtr(0, 14), emit_vtr(0, 15)),
                    lambda: emit_qload_v(4),
                    lambda: emit_qload_v(5),
                    lambda: emit_qload_v(6),
                    lambda: emit_qload_v(7),
                ] + [
                    (lambda jj=j: (emit_vtr(1, 2 * jj), emit_vtr(1, 2 * jj + 1)))
                    for j in range(8)
                ]
                emit_attention(0, inject_b0, ndefer=2,
                               defer_mid=lambda: [emit_vtr(0, j) for j in range(8)])
                emit_attention(1, [])

            # ================= P7: AllToAll attention outputs ==============
            # (per-qc staging DMAs already issued inside emit_attention)
            nc.gpsimd.collective_compute(
                "AllToAll", ALU.bypass, replica_groups=RG,
                ins=[ao_in[:].opt()], outs=[ao_out[:].opt()],
            )

            # -------- HAM keep-warm: dummy matmuls while the A2A is in flight
            with tc.tile_pool(name="ps_dummy2", bufs=1, space="PSUM") as psd2:
                dps2 = psd2.tile([128, 512], F32, tag="d2")
                for i in range(DUMMY_A2A):
                    nc.tensor.matmul(dps2[:], idn_sb[:],
                                     attnT[:, 512 * (i % 8):512 * (i % 8) + 512],
                                     start=True, stop=True)

            # ================= P8: output projection (token slice) =========
            with (
                tc.tile_pool(name="projx", bufs=1) as pxp,
                tc.tile_pool(name="ps_o", bufs=4, space="PSUM") as pso,
                tc.tile_pool(name="outp", bufs=2) as outp,
            ):
                aT = []
                for ck in range(KT8):
                    ak = pxp.tile([128, TPC], BF16, tag=f"aT{ck}", name=f"ak{ck}")
                    nc.sync.dma_start(ak[:],
                                      ao_out[128 * ck:128 * (ck + 1), :])
                    aT.append(ak)
                for tt in range(4):
                    ps0 = pso.tile([128, 512], F32, tag="po")
                    ps1 = pso.tile([128, 512], F32, tag="po")
                    for ck in range(KT8):
                        lh = aT[ck][:, 128 * tt: 128 * (tt + 1)]
                        nc.tensor.matmul(ps0[:], lh,
                                         pwt_sb[:, DIM * ck: DIM * ck + 512],
                                         start=(ck == 0), stop=(ck == KT8 - 1))
                        nc.tensor.matmul(ps1[:], lh,
                                         pwt_sb[:, DIM * ck + 512: DIM * ck + 1024],
                                         start=(ck == 0), stop=(ck == KT8 - 1))
                    ot = outp.tile([128, DIM], F32, tag="ot")
                    nc.vector.tensor_tensor(ot[:, 0:512], ps0[:],
                                            pbf_sb[:, 0:512], op=ALU.add)
                    nc.vector.tensor_tensor(ot[:, 512:1024], ps1[:],
                                            pbf_sb[:, 512:1024], op=ALU.add)
                    nc.sync.dma_start(out_dram[128 * tt:128 * (tt + 1), :], ot[:])

    nc.compile()
    return nc


def host_prep(inputs):
    x = np.asarray(inputs["x"], np.float32).reshape(TOK, DIM)
    ln_w = np.asarray(inputs["ln_w"], np.float32)
    ln_b = np.asarray(inputs["ln_b"], np.float32)
    qkv_w = np.asarray(inputs["qkv_w"], np.float32)
    qkv_b = np.asarray(inputs["qkv_b"], np.float32)
    proj_w = np.asarray(inputs["proj_w"], np.float32)
    proj_b = np.asarray(inputs["proj_b"], np.float32)

    # fold LN affine into qkv weights; fold 1/sqrt(D) into Q rows
    Wp = qkv_w * ln_w[None, :]
    bp = qkv_b + qkv_w @ ln_b
    Wp[0:DIM] *= D ** -0.5
    bp[0:DIM] *= D ** -0.5

    # destination-core-major row permutation: for core c, its 384 rows are
    # [q(h2c), q(h2c+1), k(h2c), k(h2c+1), v(h2c), v(h2c+1)]
    rows = []
    for c in range(NC):
        for blk in range(3):
            for h in (2 * c, 2 * c + 1):
                rows.extend(range(blk * DIM + h * D, blk * DIM + (h + 1) * D))
    rows = np.array(rows)
    Wperm = Wp[rows]                      # [3072, 1024]
    bperm = bp[rows]                      # [3072]

    idn = np.eye(128, dtype=np.float32).astype(BF16_NP)
    tri = (np.arange(512)[None, :] >= np.arange(128)[:, None]).astype(BF16_NP)
    emat = np.zeros((33, 128), np.float32)
    emat[0, 0:64] = 1.0
    emat[32, 64:128] = 1.0
    emat = emat.astype(BF16_NP)
    pwt = proj_w.T.copy().astype(BF16_NP)
    pbf = np.broadcast_to(proj_b.reshape(1, DIM), (128, DIM)).copy().astype(BF16_NP)
    # SBUF image per gt-tile: wt_c[128*gt+p, 128*k+o] = Wperm[128*gt+o, 128*k+p]
    wt_c = np.ascontiguousarray(
        Wperm.reshape(GT, 128, KT8, 128).transpose(0, 3, 2, 1)
        .reshape(GT * 128, DIM)).astype(BF16_NP)
    bias_c = np.ascontiguousarray(bperm.reshape(GT, 128).T)    # [128, 24]

    in_maps = []
    for c in range(NC):
        in_maps.append(dict(
            x_c=np.ascontiguousarray(x[TPC * c:TPC * (c + 1)]),
            wt_c=wt_c, bias_c=bias_c,
            pwt=pwt, pbf=pbf, idn=idn, tri=tri, emat=emat,
        ))
    return in_maps


_CACHED = {}


def kernel(**inputs) -> np.ndarray:
    _ensure_ntff_hook()
    from concourse import bass_utils
    if TRACE:
        bass_utils.upload_artifacts = lambda tmpdir: "/tmp/noupload"

    if "nc" not in _CACHED:
        _CACHED["nc"] = build_graph()
    nc = _CACHED["nc"]

    in_maps = host_prep(inputs)
    res = bass_utils.run_bass_kernel_spmd(
        nc, in_maps, core_ids=list(range(NC)), trace=TRACE,
        trace_cores=list(range(NC)) if TRACE else None)
    _CACHED["last_result"] = res
    out = np.concatenate([res.results[c]["out_c"] for c in range(NC)], axis=0)
    return out.reshape(B, T, DIM).astype(np.float32)


# revision 49
# speedup vs baseline: 1.1744x; 1.0005x over previous
"""Distributed Trainium2 Bass kernel for fused LayerNorm + causal multi-head
attention + output projection (B=2, T=2048, DIM=1024, H=16, D=64) on 8 cores.

Sharding (v6):
  - LayerNorm + QKV projection + final projection: token-parallel
    (512 tokens/core). QKV is computed on LOCAL data (full 3072-row weight)
    BEFORE any collective, so the first-collective rendezvous (launch skew)
    is absorbed by ~60us of real matmul work instead of idle waiting.
  - qkv travels via one bf16 AllToAll into head-parallel layout
    (2 heads x 2 batches per core); attention outputs return via a second
    bf16 AllToAll; projection is token-parallel again.
  - causal diagonal blocks are N-trimmed; triangular mask via precomputed
    bf16 multiply (DVE+Pool); denominators via the vnat ones-column trick.

Compute dtype: bf16 matmuls with fp32 PSUM accumulation (rel err ~5e-3).
LN affine params and the 1/sqrt(D) score scale are folded into the QKV
weights on the host.
"""
import os
import sys
import types
import numpy as np
import ml_dtypes

# ---------------------------------------------------------------- constants
B, T, DIM, D = 2, 2048, 1024, 64
H = DIM // D            # 16 heads
NC = 8                  # cores
TOK = B * T             # 4096 tokens
TPC = TOK // NC         # 512 tokens per core
KT8 = DIM // 128        # 8 contraction tiles
GT = 3 * DIM // 128     # 24 qkv output tiles of 128 rows
EPS = 1e-5

TRACE = bool(int(os.environ.get("BASS_KERNEL_TRACE", "0")))
DUMMY_QA2A = int(os.environ.get("DUMMY_QA2A", "90"))
DUMMY_A2A = int(os.environ.get("DUMMY_A2A", "60"))

BF16_NP = ml_dtypes.bfloat16


def _ensure_ntff_hook():
    """The agent image lacks antenv.axon_hooks; recreate it so trace=True works."""
    if "antenv.axon_hooks" not in sys.modules:
        mod = types.ModuleType("antenv.axon_hooks")
        mod._hook = None
        def set_axon_ntff_profile_hook(h):
            mod._hook = h
        def get_axon_ntff_profile_hook():
            return mod._hook
        mod.set_axon_ntff_profile_hook = set_axon_ntff_profile_hook
        mod.get_axon_ntff_profile_hook = get_axon_ntff_profile_hook
        sys.modules["antenv.axon_hooks"] = mod
    m = sys.modules["antenv.axon_hooks"]
    if m.get_axon_ntff_profile_hook() is None:
        try:
            from trn_agent_boot.trn_boot import _ntff_profile_via_ctypes
            m.set_axon_ntff_profile_hook(
                _ntff_profile_via_ctypes("/opt/axon/libaxon_pjrt.so"))
        except Exception:
            pass


def build_graph():
    import concourse.bass as bass
    import concourse.bacc as bacc
    import concourse.tile as tile
    import concourse.mybir as mybir

    dt = mybir.dt
    F32, BF16 = dt.float32, dt.bfloat16
    AF = mybir.ActivationFunctionType
    ALU = mybir.AluOpType
    RG = [list(range(NC))]

    nc = bacc.Bacc(None, target_bir_lowering=False, debug=False, num_devices=NC)

    # ------------------------------------------------------------ I/O
    x_in = nc.dram_tensor("x_c", [TPC, DIM], F32, kind="ExternalInput")
    wt_in = nc.dram_tensor("wt_c", [GT * 128, DIM], BF16, kind="ExternalInput")
    bias_in = nc.dram_tensor("bias_c", [128, GT], F32, kind="ExternalInput")
    pwt_in = nc.dram_tensor("pwt", [DIM, DIM], BF16, kind="ExternalInput")
    pbf_in = nc.dram_tensor("pbf", [128, DIM], BF16, kind="ExternalInput")
    idn_in = nc.dram_tensor("idn", [128, 128], BF16, kind="ExternalInput")
    tri_in = nc.dram_tensor("tri", [128, 512], BF16, kind="ExternalInput")
    emat_in = nc.dram_tensor("emat", [33, 128], BF16, kind="ExternalInput")
    out_dram = nc.dram_tensor("out_c", [TPC, DIM], F32, kind="ExternalOutput")

    with tile.TileContext(nc) as tc:
        with (
            tc.tile_pool(name="persist", bufs=1) as pers,
            tc.tile_pool(name="dram", bufs=1, space="DRAM") as dram,
        ):
            # ---------------- DRAM bounce buffers ----------------
            qa1_in = dram.tile([NC * 256, TPC], BF16)         # q+k AllToAll
            qa1_out = dram.tile([NC * 256, TPC], BF16)
            qa2_in = dram.tile([NC * 128, TPC], BF16)         # v AllToAll
            qa2_out = dram.tile([NC * 128, TPC], BF16)
            ao_in = dram.tile([NC * 128, TPC], BF16)          # attn-out AllToAll
            ao_out = dram.tile([NC * 128, TPC], BF16)

            # idn first: transposes need it early; it is tiny
            idn_sb = pers.tile([128, 128], BF16)
            nc.sync.dma_start(idn_sb[:], idn_in[:])

            # ================= P1: LayerNorm (token slice, natural) ========
            xn_sb = pers.tile([128, 4 * DIM], BF16)   # 4 token tiles side by side
            wt_sb = pers.tile([128, GT * DIM], BF16)  # gt-major, k-minor qkv weights
            with tc.tile_pool(name="ln", bufs=4) as lnp:
                # x tiles first on the DMA queue, then the 24 weight-tile DMAs
                xts = [lnp.tile([128, DIM], F32, tag="xt", name=f"xt{t}")
                       for t in range(4)]
                qk_gts = [gt for gt in range(GT) if gt % 3 != 2]
                v_gts = [gt for gt in range(GT) if gt % 3 == 2]
                wt_order = qk_gts + v_gts

                def wt_dma(gt):
                    # host pre-arranged: row block gt is the contiguous
                    # [128 partitions x 1024] SBUF image of that weight tile
                    nc.sync.dma_start(
                        wt_sb[:, DIM * gt:DIM * (gt + 1)],
                        wt_in[128 * gt:128 * (gt + 1), :])

                # interleave: first two x tiles, two weight tiles, the rest
                # of x, then the remaining weights — softens the early QKV
                # weight starvation without delaying LayerNorm's start
                nc.sync.dma_start(xts[0][:], x_in[0:128, :])
                nc.sync.dma_start(xts[1][:], x_in[128:256, :])
                wt_dma(wt_order[0])
                wt_dma(wt_order[1])
                nc.sync.dma_start(xts[2][:], x_in[256:384, :])
                nc.sync.dma_start(xts[3][:], x_in[384:512, :])
                for gt in wt_order[2:]:
                    wt_dma(gt)
                for t in range(4):
                    xt = xts[t]
                    nmu = lnp.tile([128, 1], F32, tag="nmu")
                    musum = lnp.tile([128, 1], F32, tag="musum")
                    nc.vector.reduce_sum(musum[:], xt[:], axis=mybir.AxisListType.X)
                    nc.vector.tensor_scalar_mul(nmu[:], musum[:], -1.0 / DIM)
                    sq_dump = lnp.tile([128, DIM], BF16, tag="sqd")
                    sumsq = lnp.tile([128, 1], F32, tag="sumsq")
                    nc.scalar.activation(sq_dump[:], xt[:], AF.Square,
                                         bias=nmu[:], scale=1.0,
                                         accum_out=sumsq[:])
                    vareps = lnp.tile([128, 1], F32, tag="vareps")
                    nc.vector.tensor_scalar(vareps[:], sumsq[:], 1.0 / DIM, EPS,
                                            op0=ALU.mult, op1=ALU.add)
                    std = lnp.tile([128, 1], F32, tag="std")
                    nc.scalar.activation(std[:], vareps[:], AF.Sqrt)
                    rstd = lnp.tile([128, 1], F32, tag="rstd")
                    nc.vector.reciprocal(rstd[:], std[:])
                    nmr = lnp.tile([128, 1], F32, tag="nmr")
                    nc.vector.scalar_tensor_tensor(
                        nmr[:], nmu[:], 1.0, rstd[:],
                        op0=ALU.mult, op1=ALU.mult)
                    nc.scalar.activation(xn_sb[:, DIM * t:DIM * (t + 1)], xt[:],
                                         AF.Identity, bias=nmr[:], scale=rstd[:])

            # ================= P2: transpose xn -> xnT =====================
            xnT_sb = pers.tile([128, KT8 * TPC], BF16)  # [dim-tile partition, k*512+t128]
            with tc.tile_pool(name="ps_tr", bufs=6, space="PSUM") as pstr:
                for t in range(4):
                    for k in range(KT8):
                        trp = pstr.tile([128, 128], BF16, tag="tr")
                        nc.tensor.transpose(
                            trp[:], xn_sb[:, DIM * t + 128 * k: DIM * t + 128 * (k + 1)],
                            idn_sb[:])
                        nc.vector.tensor_copy(
                            xnT_sb[:, TPC * k + 128 * t: TPC * k + 128 * (t + 1)],
                            trp[:])

            # ---------------- other weight loads (background) -------------
            bias_sb = pers.tile([128, GT], F32)
            nc.sync.dma_start(bias_sb[:], bias_in[:])
            pwt_sb = pers.tile([128, KT8 * DIM], BF16)      # k-major proj weights
            nc.sync.dma_start(
                pwt_sb[:].rearrange("p (k o) -> p k o", o=DIM),
                pwt_in[:].rearrange("(k p) o -> p k o", p=128),
            )
            pbf_sb = pers.tile([128, DIM], BF16)
            nc.sync.dma_start(pbf_sb[:], pbf_in[:])
            tri_sb = pers.tile([128, 512], BF16)
            nc.sync.dma_start(tri_sb[:], tri_in[:])
            emat_sb = pers.tile([33, 128], BF16)
            nc.sync.dma_start(emat_sb[:], emat_in[:])
            sums_col = pers.tile([33, 512], F32)
            nc.vector.memset(sums_col[:], 1.0)

            # ================= P3: local token-parallel QKV ================
            # All 3072 qkv rows for this core's 512 tokens; rows are ordered
            # destination-core-major on the host, so row block 128*gt is the
            # (gt%3)-th third of chunk r=gt//3 of the AllToAll input.
            qkvL = pers.tile([128, GT * TPC], BF16)
            with tc.tile_pool(name="ps_q", bufs=3, space="PSUM") as psq:
                def emit_qkv(gt):
                    psg = psq.tile([128, TPC], F32, tag="q")
                    for k in range(KT8):
                        nc.tensor.matmul(
                            psg[:],
                            wt_sb[:, DIM * gt + 128 * k: DIM * gt + 128 * (k + 1)],
                            xnT_sb[:, TPC * k:TPC * (k + 1)],
                            start=(k == 0), stop=(k == KT8 - 1))
                    nc.vector.tensor_scalar(
                        qkvL[:, TPC * gt:TPC * (gt + 1)], psg[:],
                        bias_sb[:, gt:gt + 1], None, op0=ALU.add)
                    r, c = gt // 3, gt % 3
                    if c == 2:
                        dst = qa2_in[128 * r:128 * (r + 1), :]
                    else:
                        dst = qa1_in[256 * r + 128 * c: 256 * r + 128 * (c + 1), :]
                    nc.sync.dma_start(dst, qkvL[:, TPC * gt:TPC * (gt + 1)])

                # q+k tiles first -> AllToAll #1 overlaps the v tiles' matmuls
                for gt in qk_gts:
                    emit_qkv(gt)
                nc.gpsimd.collective_compute(
                    "AllToAll", ALU.bypass, replica_groups=RG,
                    ins=[qa1_in[:].opt()], outs=[qa1_out[:].opt()],
                )
                for gt in v_gts:
                    emit_qkv(gt)
                nc.gpsimd.collective_compute(
                    "AllToAll", ALU.bypass, replica_groups=RG,
                    ins=[qa2_in[:].opt()], outs=[qa2_out[:].opt()],
                )

            # -------- HAM keep-warm: dummy matmuls while the A2As are in flight
            with tc.tile_pool(name="ps_dummy", bufs=1, space="PSUM") as psd:
                dps = psd.tile([128, 512], F32, tag="d")
                for i in range(DUMMY_QA2A):
                    nc.tensor.matmul(dps[:], idn_sb[:],
                                     xnT_sb[:, 512 * (i % 8):512 * (i % 8) + 512],
                                     start=True, stop=True)

            # ================= P5/P6: head-parallel attention ==============
            qkvT = []
            for name in ("qT", "kT", "vT"):
                t_ = pers.tile([128, TOK], BF16, name=name)
                qkvT.append(t_)
            qT_sb, kT_sb, vT_sb = qkvT
            vnat = []
            for b in range(B):
                vb = pers.tile([128, 16 * 130], BF16, name=f"vnat{b}")
                nc.vector.memset(
                    vb[:].rearrange("p (j a w) -> p j a w", a=2, w=65)[:, :, :, 64:65], 1.0)
                vnat.append(vb)
            attnT = pers.tile([128, TOK], BF16)

            with (
                tc.tile_pool(name="pt", bufs=14) as ptp,
                tc.tile_pool(name="ps_s", bufs=3, space="PSUM") as pss,
                tc.tile_pool(name="ps_pv", bufs=4, space="PSUM") as psp,
                tc.tile_pool(name="ps_bc", bufs=1, space="PSUM") as psb,
                tc.tile_pool(name="sm", bufs=2) as smp,
            ):
                def emit_qload_qk(s):
                    # my-heads q/k for source-core s's 512 tokens (A2A#1)
                    nc.sync.dma_start(qT_sb[:, TPC * s:TPC * (s + 1)],
                                      qa1_out[256 * s: 256 * s + 128, :])
                    nc.sync.dma_start(kT_sb[:, TPC * s:TPC * (s + 1)],
                                      qa1_out[256 * s + 128: 256 * s + 256, :])

                def emit_qload_v(s):
                    # my-heads v (A2A#2) — issued only after all needed qk
                    # loads so its wait does not clog the DMA queue
                    nc.sync.dma_start(vT_sb[:, TPC * s:TPC * (s + 1)],
                                      qa2_out[128 * s: 128 * s + 128, :])

                def emit_vtr(b, j):
                    vtr = psb.tile([128, 128], BF16, tag="bc")
                    nc.tensor.transpose(
                        vtr[:],
                        vT_sb[:, b * T + 128 * j: b * T + 128 * (j + 1)],
                        idn_sb[:])
                    nc.vector.tensor_copy(
                        vnat[b][:, 130 * j: 130 * j + 64], vtr[:, 0:64])
                    nc.vector.tensor_copy(
                        vnat[b][:, 130 * j + 65: 130 * j + 129], vtr[:, 64:128])

                def emit_attention(b, inject, ndefer=0, defer_mid=None):
                    ii = 0
                    deferred = []
                    for qc in range(4):
                        q0 = b * T + 512 * qc
                        r = 4 * b + qc
                        pvA = psp.tile([65, 512], F32, tag="pv")
                        pvB = psp.tile([65, 512], F32, tag="pv")
                        nkp = 4 * qc + 4
                        # defer PV for the first ndefer chunks: their QK/exp
                        # streams then run in the shadow of the v AllToAll with
                        # no vnat-dependent PE op blocking the in-order queue.
                        defer = qc < ndefer
                        pend = None
                        pends = []

                        def flush(p4, stop, pvA=pvA, pvB=pvB):
                            pkp, ppA, ppB, po = p4
                            nc.tensor.matmul(pvA[:, po:512],
                                             vnat[b][:, 130 * pkp:130 * pkp + 65],
                                             ppA[:, po:512],
                                             start=(pkp == 0), stop=stop,
                                             skip_group_check=True)
                            nc.tensor.matmul(pvB[:, po:512],
                                             vnat[b][:, 130 * pkp + 65:130 * pkp + 130],
                                             ppB[:, po:512],
                                             start=(pkp == 0), stop=stop,
                                             skip_group_check=True)

                        def norm(pvA=pvA, pvB=pvB, q0=q0, r=r):
                            nc.vector.tensor_copy(sums_col[0:1, :], pvA[64:65, :])
                            nc.vector.tensor_copy(sums_col[32:33, :], pvB[64:65, :])
                            rec = smp.tile([33, 512], F32, tag="rec")
                            nc.vector.reciprocal_approx_fast(rec[:], sums_col[:])
                            recb = smp.tile([33, 512], BF16, tag="recb")
                            nc.vector.tensor_copy(recb[:], rec[:])
                            bc2 = psb.tile([128, 512], F32, tag="bc")
                            nc.tensor.matmul(bc2[:], emat_sb[:], recb[:],
                                             start=True, stop=True)
                            bc2s = smp.tile([128, 512], BF16, tag="bc2s")
                            nc.vector.tensor_copy(bc2s[:], bc2[:])
                            nc.vector.tensor_tensor(
                                attnT[0:64, q0:q0 + 512], pvA[0:64, :],
                                bc2s[0:64, :], op=ALU.mult)
                            nc.vector.tensor_tensor(
                                attnT[64:128, q0:q0 + 512], pvB[0:64, :],
                                bc2s[64:128, :], op=ALU.mult)
                            nc.sync.dma_start(ao_in[128 * r:128 * (r + 1), :],
                                              attnT[:, TPC * r:TPC * (r + 1)])

                        for kp in range(nkp):
                            k0 = b * T + 128 * kp
                            j = kp - 4 * qc       # >=0 -> diagonal block
                            o = 128 * j if j > 0 else 0
                            n = 512 - o
                            sA = pss.tile([128, 512], F32, tag="s")
                            sB = pss.tile([128, 512], F32, tag="s")
                            nc.tensor.matmul(sA[:, o:512], kT_sb[0:64, k0:k0 + 128],
                                             qT_sb[0:64, q0 + o:q0 + 512],
                                             start=True, stop=True)
                            nc.tensor.matmul(sB[:, o:512], kT_sb[64:128, k0:k0 + 128],
                                             qT_sb[64:128, q0 + o:q0 + 512],
                                             start=True, stop=True)
                            if not defer and pend is not None:
                                flush(pend, stop=False)
                            pA = ptp.tile([128, 512], BF16, tag="pA")
                            pB = ptp.tile([128, 512], BF16, tag="pB")
                            nc.scalar.activation(pA[:, o:512], sA[:, o:512], AF.Exp)
                            nc.scalar.activation(pB[:, o:512], sB[:, o:512], AF.Exp)
                            if j >= 0:
                                nc.vector.tensor_tensor(
                                    pA[:, o:512], pA[:, o:512], tri_sb[:, 0:n],
                                    op=ALU.mult)
                                nc.gpsimd.tensor_tensor(
                                    pB[:, o:512], pB[:, o:512], tri_sb[:, 0:n],
                                    op=ALU.mult)
                            if defer:
                                pends.append((kp, pA, pB, o))
                            else:
                                pend = (kp, pA, pB, o)
                            if ii < len(inject):
                                inject[ii]()
                                ii += 1
                        if defer:
                            deferred.append((flush, pends, norm))
                            if qc == ndefer - 1:
                                defer_mid()   # V transposes, now that QK is queued
                                for dflush, dpends, dnorm in deferred:
                                    for idx, p4 in enumerate(dpends):
                                        dflush(p4, stop=(idx == len(dpends) - 1))
                                    dnorm()
                        else:
                            flush(pend, stop=True)
                            norm()
                    while ii < len(inject):
                        inject[ii]()
                        ii += 1

                emit_qload_qk(0)
                emit_qload_qk(1)
                inject_b0 = [
                    # qc0+qc1 slots (12, DMA-only so the deferred QK stream
                    # is never blocked by a vnat-dependent PE op; v loads
                    # queue after all early qk loads)
                    lambda: emit_qload_qk(2),
                    lambda: emit_qload_qk(3),
                    lambda: emit_qload_v(0),
                    lambda: emit_qload_v(1),
                    lambda: emit_qload_v(2),
                    lambda: emit_qload_v(3),
                    lambda: emit_qload_qk(4),
                    lambda: emit_qload_qk(5),
                    lambda: emit_qload_qk(6),
                    lambda: emit_qload_qk(7),
                    lambda: None,
                    lambda: None,
                    # qc2 slots (12)
                    lambda: (emit_vtr(0, 8), emit_vtr(0, 9)),
                    lambda: (emit_vtr(0, 10), emit_vtr(0, 11)),
                    lambda: (emit_vtr(0, 12), emit_vtr(0, 13)),
                    lambda: (emit_vtr(0, 14), emit_vtr(0, 15)),
                    lambda: emit_qload_v(4),
                    lambda: emit_qload_v(5),
                    lambda: emit_qload_v(6),
                    lambda: emit_qload_v(7),
                ] + [
                    (lambda jj=j: (emit_vtr(1, 2 * jj), emit_vtr(1, 2 * jj + 1)))
                    for j in range(8)
                ]
                emit_attention(0, inject_b0, ndefer=2,
                               defer_mid=lambda: [emit_vtr(0, j) for j in range(8)])
                emit_attention(1, [])

            # ================= P7: AllToAll attention outputs ==============
            # (per-qc staging DMAs already issued inside emit_attention)
            nc.gpsimd.collective_compute(
                "AllToAll", ALU.bypass, replica_groups=RG,
                ins=[ao_in[:].opt()], outs=[ao_out[:].opt()],
            )

            # -------- HAM keep-warm: dummy matmuls while the A2A is in flight
            with tc.tile_pool(name="ps_dummy2", bufs=1, space="PSUM") as psd2:
                dps2 = psd2.tile([128, 512], F32, tag="d2")
                for i in range(DUMMY_A2A):
                    nc.tensor.matmul(dps2[:], idn_sb[:],
                                     attnT[:, 512 * (i % 8):512 * (i % 8) + 512],
                                     start=True, stop=True)

            # ================= P8: output projection (token slice) =========
            with (
                tc.tile_pool(name="projx", bufs=1) as pxp,
                tc.tile_pool(name="ps_o", bufs=4, space="PSUM") as pso,
                tc.tile_pool(name="outp", bufs=2) as outp,
            ):
                aT = []
                for ck in range(KT8):
                    ak = pxp.tile([128, TPC], BF16, tag=f"aT{ck}", name=f"ak{ck}")
                    nc.sync.dma_start(ak[:],
                                      ao_out[128 * ck:128 * (ck + 1), :])
                    aT.append(ak)
                for tt in range(4):
                    ps0 = pso.tile([128, 512], F32, tag="po")
                    ps1 = pso.tile([128, 512], F32, tag="po")
                    for ck in range(KT8):
                        lh = aT[ck][:, 128 * tt: 128 * (tt + 1)]
                        nc.tensor.matmul(ps0[:], lh,
                                         pwt_sb[:, DIM * ck: DIM * ck + 512],
                                         start=(ck == 0), stop=(ck == KT8 - 1))
                        nc.tensor.matmul(ps1[:], lh,
                                         pwt_sb[:, DIM * ck + 512: DIM * ck + 1024],
                                         start=(ck == 0), stop=(ck == KT8 - 1))
                    ot = outp.tile([128, DIM], F32, tag="ot")
                    nc.vector.tensor_tensor(ot[:, 0:512], ps0[:],
                                            pbf_sb[:, 0:512], op=ALU.add)
                    nc.vector.tensor_tensor(ot[:, 512:1024], ps1[:],
                                            pbf_sb[:, 512:1024], op=ALU.add)
                    nc.sync.dma_start(out_dram[128 * tt:128 * (tt + 1), :], ot[:])

    nc.compile()
    return nc


def host_prep(inputs):
    x = np.asarray(inputs["x"], np.float32).reshape(TOK, DIM)
    ln_w = np.asarray(inputs["ln_w"], np.float32)
    ln_b = np.asarray(inputs["ln_b"], np.float32)
    qkv_w = np.asarray(inputs["qkv_w"], np.float32)
    qkv_b = np.asarray(inputs["qkv_b"], np.float32)
    proj_w = np.asarray(inputs["proj_w"], np.float32)
    proj_b = np.asarray(inputs["proj_b"], np.float32)

    # fold LN affine into qkv weights; fold 1/sqrt(D) into Q rows
    Wp = qkv_w * ln_w[None, :]
    bp = qkv_b + qkv_w @ ln_b
    Wp[0:DIM] *= D ** -0.5
    bp[0:DIM] *= D ** -0.5

    # destination-core-major row permutation: for core c, its 384 rows are
    # [q(h2c), q(h2c+1), k(h2c), k(h2c+1), v(h2c), v(h2c+1)]
    rows = []
    for c in range(NC):
        for blk in range(3):
            for h in (2 * c, 2 * c + 1):
                rows.extend(range(blk * DIM + h * D, blk * DIM + (h + 1) * D))
    rows = np.array(rows)
    Wperm = Wp[rows]                      # [3072, 1024]
    bperm = bp[rows]                      # [3072]

    idn = np.eye(128, dtype=np.float32).astype(BF16_NP)
    tri = (np.arange(512)[None, :] >= np.arange(128)[:, None]).astype(BF16_NP)
    emat = np.zeros((33, 128), np.float32)
    emat[0, 0:64] = 1.0
    emat[32, 64:128] = 1.0
    emat = emat.astype(BF16_NP)
    pwt = proj_w.T.copy().astype(BF16_NP)
    pbf = np.broadcast_to(proj_b.reshape(1, DIM), (128, DIM)).copy().astype(BF16_NP)
    # SBUF image per gt-tile: wt_c[128*gt+p, 128*k+o] = Wperm[128*gt+o, 128*k+p]
    wt_c = np.ascontiguousarray(
        Wperm.reshape(GT, 128, KT8, 128).transpose(0, 3, 2, 1)
        .reshape(GT * 128, DIM)).astype(BF16_NP)
    bias_c = np.ascontiguousarray(bperm.reshape(GT, 128).T)    # [128, 24]

    in_maps = []
    for c in range(NC):
        in_maps.append(dict(
            x_c=np.ascontiguousarray(x[TPC * c:TPC * (c + 1)]),
            wt_c=wt_c, bias_c=bias_c,
            pwt=pwt, pbf=pbf, idn=idn, tri=tri, emat=emat,
        ))
    return in_maps


_CACHED = {}


def kernel(**inputs) -> np.ndarray:
    _ensure_ntff_hook()
    from concourse import bass_utils
    if TRACE:
        bass_utils.upload_artifacts = lambda tmpdir: "/tmp/noupload"

    if "nc" not in _CACHED:
        _CACHED["nc"] = build_graph()
    nc = _CACHED["nc"]

    in_maps = host_prep(inputs)
    res = bass_utils.run_bass_kernel_spmd(
        nc, in_maps, core_ids=list(range(NC)), trace=TRACE,
        trace_cores=list(range(NC)) if TRACE else None)
    _CACHED["last_result"] = res
    out = np.concatenate([res.results[c]["out_c"] for c in range(NC)], axis=0)
    return out.reshape(B, T, DIM).astype(np.float32)
